# revision 2
# baseline (speedup 1.0000x reference)
"""Trainium2 Bass kernel for a dense transformer block (B=4, T=2048, D=1024, H=16).

Sharding v2: 8 cores = 4 batches x 2 token-parities; no collectives.
Core (b, par) owns query tiles {0,3} (par=0) or {1,2} (par=1) of 512 tokens
(both = 5 causal units, balanced).  Every core computes LN1 + k/v projections
for all 4 key tiles of its batch (redundant within a pair), q for its own
tiles, full causal attention + complete Wo for its own tokens, then LN2 + MLP
locally.  The compiled program is identical on all cores (SPMD); per-core
differences live purely in DRAM data.

Uniformity trick: the host stages the four 512-token x tiles in per-core SLOT
order (par=0: tiles [1,0,2,3]; par=1: [0,1,3,2]) so that the two own q tiles
always sit at slots 1 and 3.  k/v are computed per slot; attention pass0
(q = slot1) runs over key blocks 0..7 (slots 0-1) and pass1 (q = slot3) over
blocks 0..15, with per-core causal masks (DRAM data) zeroing whatever a core
must not see.  Diagonal blocks (pass0: 4..7, pass1: 12..15 — the diag slot on
both cores) process only columns >= 128*j (exact shrink).

All matmul operands are bf16 (fp32 PSUM).  LayerNorm gain + mean subtraction
(rank-1) are folded into weights host-side; rstd is applied to the layer
input (xs = x * rstd) so projection epilogues are one bias-add+convert pass.
"""

import os
import sys

for _p in ("/opt/trn_rl_repo", "/root/.axon_site/_ro/trn_rl_repo"):
    if os.path.isdir(_p) and _p not in sys.path:
        sys.path.append(_p)

import ml_dtypes
import numpy as np

import concourse.bass as bass
import concourse.tile as tile
from concourse import bacc, mybir
from concourse.bass_utils import run_bass_kernel_spmd

AF = mybir.ActivationFunctionType
ALU = mybir.AluOpType
FP32 = mybir.dt.float32
BF16 = mybir.dt.bfloat16
NPBF = ml_dtypes.bfloat16

B, T, D, H = 4, 2048, 1024, 16
HD = D // H          # 64
DFF = 4 * D          # 4096
P = 128
DK = D // P          # 8   D k-tiles
NT = T // 512        # 4   512-token tiles
NP = H // 2          # 8   head pairs
FFT = DFF // P       # 32  DFF tiles
TC = T // P          # 16  128-token chunks
EPS = 1e-5
SCALE = 1.0 / 8.0    # 1/sqrt(HD)
NOWN = 2             # own 512-token tiles per core
NBLK = (8, 16)       # key blocks per attention pass
Q_SLOTS = (1, 3)     # slots holding the own q tiles

# slot -> tile permutations per parity
SLOT_TILES = ([1, 0, 2, 3], [0, 1, 3, 2])


def _shrink(pi, kt):
    if pi == 0 and kt >= 4:
        return 128 * (kt - 4)
    if pi == 1 and kt >= 12:
        return 128 * (kt - 12)
    return 0


def _masked(pi, kt):
    return kt >= 8 if pi == 1 else True


def build_program(debug=False, phases=("att", "mlp")):
    nc = bacc.Bacc("TRN2", target_bir_lowering=False, debug=False)

    # ---- DRAM I/O ----
    xbf = nc.dram_tensor("xbf", [P, NT, DK, 512], BF16, kind="ExternalInput")
    xf32 = nc.dram_tensor("xf32", [P, NOWN, DK, 512], FP32, kind="ExternalInput")
    wk = nc.dram_tensor("wk", [DK, P, DK, P], BF16, kind="ExternalInput")
    wv = nc.dram_tensor("wv", [P, DK, D], BF16, kind="ExternalInput")
    wq = nc.dram_tensor("wq", [NP, P, DK, P], BF16, kind="ExternalInput")
    wo = nc.dram_tensor("wo", [DK, P, NP, P], BF16, kind="ExternalInput")
    w1 = nc.dram_tensor("w1", [FFT, P, DK, P], BF16, kind="ExternalInput")
    w2 = nc.dram_tensor("w2", [DK, P, FFT, P], BF16, kind="ExternalInput")
    masks = nc.dram_tensor("masks", [2, P, 8, 512], BF16, kind="ExternalInput")
    ckq = nc.dram_tensor("ckq", [P, 2 * DK], FP32, kind="ExternalInput")  # [ck|cq]
    cvb = nc.dram_tensor("cvb", [P, D], FP32, kind="ExternalInput")
    bo = nc.dram_tensor("bo", [P, DK], FP32, kind="ExternalInput")
    c1 = nc.dram_tensor("c1", [P, FFT], FP32, kind="ExternalInput")
    b2 = nc.dram_tensor("b2", [P, DK], FP32, kind="ExternalInput")
    out = nc.dram_tensor("out", [P, NOWN, DK, 512], FP32, kind="ExternalOutput")
    if debug:
        dbg_kT = nc.dram_tensor("dbg_kT", [P, NP, T], BF16, kind="ExternalOutput")
        dbg_v = nc.dram_tensor("dbg_v", [P, H, TC, HD + 1], BF16,
                               kind="ExternalOutput")
        dbg_q = nc.dram_tensor("dbg_q", [P, NOWN, NP, 512], BF16,
                               kind="ExternalOutput")
        dbg_x2 = nc.dram_tensor("dbg_x2", [P, NOWN, DK, 512], BF16,
                                kind="ExternalOutput")
        dbg_y = nc.dram_tensor("dbg_y", [P, NOWN, NP, 512], BF16,
                               kind="ExternalOutput")

    with tile.TileContext(nc) as tc:
        with (
            tc.tile_pool(name="persist", bufs=1) as persist,
            tc.tile_pool(name="psum", bufs=1, space="PSUM") as psum,
        ):
            # ---- persistent small tiles ----
            ones_bf = persist.tile([P, 1], BF16)
            nc.vector.memset(ones_bf, 1.0)
            ckq_sb = persist.tile([P, 2 * DK], FP32)
            nc.sync.dma_start(ckq_sb, ckq[:, :])
            cvb_sb = persist.tile([P, D], FP32)
            nc.sync.dma_start(cvb_sb, cvb[:, :])
            bo_sb = persist.tile([P, DK], FP32)
            nc.sync.dma_start(bo_sb, bo[:, :])
            c1_sb = persist.tile([P, FFT], FP32)
            nc.sync.dma_start(c1_sb, c1[:, :])
            b2_sb = persist.tile([P, DK], FP32)
            nc.sync.dma_start(b2_sb, b2[:, :])
            eps_sb = persist.tile([1, 1], FP32)
            nc.vector.memset(eps_sb, EPS)
            row_a = persist.tile([1, 512], FP32)
            row_b = persist.tile([1, 512], FP32)
            row_mu = persist.tile([1, 512], FP32)
            x2_sb = persist.tile([P, NOWN, DK, 512], BF16)
            x2s_sb = persist.tile([P, NOWN, DK, 512], BF16)

            def ln_rows(s_ps, q_ps, rs_bf):
                nc.vector.tensor_scalar(row_mu, s_ps, 1.0 / D, None, ALU.mult)
                nc.vector.tensor_scalar(row_a, q_ps, 1.0 / D, None, ALU.mult)
                nc.vector.tensor_mul(row_b, row_mu, row_mu)
                nc.vector.tensor_sub(row_a, row_a, row_b)
                nc.scalar.activation(row_b, row_a, AF.Sqrt, bias=eps_sb)
                with nc.allow_low_precision(reason="bf16 rstd by design"):
                    nc.vector.reciprocal(rs_bf, row_b)

            with (
                tc.tile_pool(name="attn", bufs=1) as attn,
                tc.tile_pool(name="work", bufs=1) as work,
            ):
                wv_sb = attn.tile([P, DK, D], BF16)
                nc.sync.dma_start(wv_sb, wv[:, :, :])
                kT_sb = attn.tile([P, NP, T], BF16)        # [64*hb+e, pair, ktok]
                v_sb = attn.tile([P, H, TC, HD + 1], BF16)  # [ktok%128, h, kchunk, e|1]
                nc.vector.memset(v_sb[:, :, :, HD:HD + 1], 1.0)
                q_sb = attn.tile([P, NOWN, NP, 512], BF16)
                ysb = attn.tile([P, NP, 512], BF16)

                def emit_den(y_ps, pt):
                    """Normalize y_ps into ysb[:, pt, :] (reciprocal of the
                    ones-column denominator, broadcast, multiply)."""
                    for hb in range(2):
                        den = work.tile([HD + 1, 512], FP32, tag="den", bufs=2)
                        nc.vector.reciprocal(den[HD:HD + 1, :],
                                             y_ps[hb][HD:HD + 1, :])
                        rec = work.tile([1, 512], FP32, tag="rec", bufs=2)
                        nc.sync.dma_start(rec, den[HD:HD + 1, :])
                        rb = work.tile([HD, 512], FP32, tag="rb", bufs=2)
                        nc.gpsimd.partition_broadcast(rb, rec)
                        if hb == 0:
                            nc.vector.tensor_mul(ysb[0:HD, pt, :],
                                                 y_ps[hb][0:HD, :], rb)
                        else:
                            yst = work.tile([HD, 512], BF16, tag="yst", bufs=1)
                            nc.vector.tensor_mul(yst, y_ps[hb][0:HD, :], rb)
                            nc.sync.dma_start(ysb[HD:2 * HD, pt, :], yst)

                def attention(pi):
                    nblk = NBLK[pi]
                    mask_sb = work.tile([P, 8, 512], BF16, tag="mask", bufs=1)
                    nc.sync.dma_start(mask_sb, masks[pi])
                    prev_y = None
                    for pt in range(NP):
                        y_ps = [psum.tile([HD + 1, 512], FP32, name=f"yps{hb}",
                                          tag="y", bufs=2) for hb in range(2)]
                        for kt in range(nblk):
                            off = _shrink(pi, kt)
                            pexp = []
                            for hb in range(2):
                                hsl = slice(hb * HD, (hb + 1) * HD)
                                s_ps = psum.tile([P, 512], FP32, tag="mm", bufs=2)
                                nc.tensor.matmul(
                                    s_ps[:, off:],
                                    kT_sb[hsl, pt, kt * P:(kt + 1) * P],
                                    q_sb[hsl, pi, pt, off:], start=True, stop=True)
                                pe = work.tile([P, 512], BF16, tag="pexp", bufs=3)
                                nc.scalar.activation(pe[:, off:], s_ps[:, off:],
                                                     AF.Exp, scale=SCALE)
                                if _masked(pi, kt):
                                    mi = kt if pi == 0 else kt - 8
                                    nc.vector.tensor_mul(pe[:, off:], pe[:, off:],
                                                         mask_sb[:, mi, off:])
                                pexp.append(pe)
                            # normalize the previous pair's y while this one's
                            # exp/AV chain runs (keeps the DVE den chain out of
                            # the critical path)
                            if kt == 1 and prev_y is not None:
                                emit_den(prev_y, pt - 1)
                            for hb in range(2):
                                nc.tensor.matmul(
                                    y_ps[hb][:, off:], v_sb[:, 2 * pt + hb, kt, :],
                                    pexp[hb][:, off:],
                                    start=(kt == 0), stop=(kt == nblk - 1))
                        prev_y = y_ps
                    emit_den(prev_y, NP - 1)
                    if debug:
                        nc.sync.dma_start(dbg_y[:, pi], ysb)
                    # ---- Wo + x2 + LN2 stats ----
                    s2_ps = psum.tile([1, 512], FP32, tag="st", bufs=2)
                    q2_ps = psum.tile([1, 512], FP32, tag="st", bufs=2)
                    for ot in range(DK):
                        wo_blk = work.tile([P, NP, P], BF16, tag="wob", bufs=2)
                        nc.sync.dma_start(wo_blk, wo[ot])
                        xo_blk = work.tile([P, 512], FP32, tag="xob", bufs=1)
                        nc.sync.dma_start(xo_blk, xf32[:, pi, ot, :])
                        pp = psum.tile([P, 512], FP32, tag="mm", bufs=2)
                        for pt in range(NP):
                            nc.tensor.matmul(pp, wo_blk[:, pt, :], ysb[:, pt, :],
                                             start=(pt == 0), stop=(pt == NP - 1))
                        nc.vector.scalar_tensor_tensor(
                            x2_sb[:, pi, ot, :], pp, bo_sb[:, ot:ot + 1], xo_blk,
                            ALU.add, ALU.add)
                        nc.tensor.matmul(s2_ps, ones_bf, x2_sb[:, pi, ot, :],
                                         start=(ot == 0), stop=(ot == DK - 1))
                        xsq = work.tile([P, 512], BF16, tag="xsq", bufs=1)
                        nc.vector.tensor_mul(xsq, x2_sb[:, pi, ot, :],
                                             x2_sb[:, pi, ot, :])
                        nc.tensor.matmul(q2_ps, ones_bf, xsq,
                                         start=(ot == 0), stop=(ot == DK - 1))
                    rs2_bf = work.tile([1, 512], BF16, tag="rsrow", bufs=2)
                    ln_rows(s2_ps, q2_ps, rs2_bf)
                    rsb2 = work.tile([P, 512], BF16, tag="rsb", bufs=2)
                    nc.gpsimd.partition_broadcast(rsb2, rs2_bf)
                    for kt in range(DK):
                        nc.vector.tensor_mul(x2s_sb[:, pi, kt, :],
                                             x2_sb[:, pi, kt, :], rsb2)

                # ==== per 512-token slot: LN1 stats, k/v (+q); attention ====
                def load_and_stats(tt):
                    """DMA x tile, LN1 stats, then scale it in place by rstd."""
                    xt = work.tile([P, DK, 512], BF16, tag="xt", bufs=2)
                    nc.sync.dma_start(xt, xbf[:, tt])
                    s_ps = psum.tile([1, 512], FP32, tag="st", bufs=2)
                    q_ps = psum.tile([1, 512], FP32, tag="st", bufs=2)
                    for kt in range(DK):
                        nc.tensor.matmul(s_ps, ones_bf, xt[:, kt, :],
                                         start=(kt == 0), stop=(kt == DK - 1))
                    for kt in range(DK):
                        xsq = work.tile([P, 512], BF16, tag="xsq", bufs=1)
                        nc.vector.tensor_mul(xsq, xt[:, kt, :], xt[:, kt, :])
                        nc.tensor.matmul(q_ps, ones_bf, xsq,
                                         start=(kt == 0), stop=(kt == DK - 1))
                    rs_bf = work.tile([1, 512], BF16, tag="rsrow", bufs=2)
                    ln_rows(s_ps, q_ps, rs_bf)
                    rsb = work.tile([P, 512], BF16, tag="rsb", bufs=2)
                    nc.gpsimd.partition_broadcast(rsb, rs_bf)
                    for kt in range(DK):
                        nc.vector.tensor_mul(xt[:, kt, :], xt[:, kt, :], rsb)
                    return xt

                xs = load_and_stats(0)
                for tt in range(NT):
                    ts5 = slice(tt * 512, (tt + 1) * 512)
                    # k-proj (all 16 heads)
                    for ot in range(DK):
                        wk_blk = work.tile([P, DK, P], BF16, tag="wkb", bufs=2)
                        nc.sync.dma_start(wk_blk, wk[ot])
                        pp = psum.tile([P, 512], FP32, tag="mm", bufs=2)
                        for kt in range(DK):
                            nc.tensor.matmul(pp, wk_blk[:, kt, :], xs[:, kt, :],
                                             start=(kt == 0), stop=(kt == DK - 1))
                        nc.vector.tensor_scalar(kT_sb[:, ot, ts5], pp,
                                                ckq_sb[:, ot:ot + 1], None, ALU.add)
                    # next tile's stats run here so the DVE/broadcast chain
                    # hides under this tile's remaining projections
                    xs_next = load_and_stats(tt + 1) if tt + 1 < NT else None
                    # v-proj (token-major: x chunk stationary, Wv moving)
                    for st in range(4):
                        pv = [psum.tile([P, 512], FP32, name=f"pv{h}", tag="mm",
                                        bufs=2) for h in range(2)]
                        for kt in range(DK):
                            xsl = xs[:, kt, st * P:(st + 1) * P]
                            for half in range(2):
                                nc.tensor.matmul(
                                    pv[half], xsl,
                                    wv_sb[:, kt, half * 512:(half + 1) * 512],
                                    start=(kt == 0), stop=(kt == DK - 1))
                        for half in range(2):
                            dest = v_sb[:, half * NP:(half + 1) * NP,
                                        tt * 4 + st, 0:HD]
                            nc.vector.tensor_tensor(
                                dest,
                                pv[half].rearrange("p (h e) -> p h e", h=NP),
                                cvb_sb[:, half * 512:(half + 1) * 512]
                                .rearrange("p (h e) -> p h e", h=NP), ALU.add)
                    # q-proj on own slots
                    if tt in Q_SLOTS:
                        oi = Q_SLOTS.index(tt)
                        for ot in range(NP):
                            wq_blk = work.tile([P, DK, P], BF16, tag="wqb", bufs=2)
                            nc.sync.dma_start(wq_blk, wq[ot])
                            pp = psum.tile([P, 512], FP32, tag="mm", bufs=2)
                            for kt in range(DK):
                                nc.tensor.matmul(pp, wq_blk[:, kt, :], xs[:, kt, :],
                                                 start=(kt == 0),
                                                 stop=(kt == DK - 1))
                            nc.vector.tensor_scalar(
                                q_sb[:, oi, ot, :], pp,
                                ckq_sb[:, DK + ot:DK + ot + 1], None, ALU.add)
                        if "att" in phases:
                            attention(oi)
                    xs = xs_next

                if debug:
                    nc.sync.dma_start(dbg_kT[:, :, :], kT_sb)
                    nc.sync.dma_start(dbg_v[:, :, :, :], v_sb)
                    nc.sync.dma_start(dbg_q[:, :, :, :], q_sb)
                    nc.sync.dma_start(dbg_x2[:, :, :, :], x2_sb)

            # ==== MLP over both own tiles ====
            with tc.tile_pool(name="mlp", bufs=1) as mlp:
              if "mlp" in phases:
                m_sb = mlp.tile([P, NOWN, FFT, 512], BF16)
                for fft in range(FFT):
                    w1_blk = mlp.tile([P, DK, P], BF16, tag="w1b", bufs=3)
                    nc.sync.dma_start(w1_blk, w1[fft])
                    for oi in range(NOWN):
                        pp = psum.tile([P, 512], FP32, tag="mm", bufs=2)
                        for kt in range(DK):
                            nc.tensor.matmul(pp, w1_blk[:, kt, :],
                                             x2s_sb[:, oi, kt, :],
                                             start=(kt == 0), stop=(kt == DK - 1))
                        nc.scalar.activation(m_sb[:, oi, fft, :], pp, AF.Gelu,
                                             bias=c1_sb[:, fft:fft + 1])
                for ot in range(DK):
                    w2_blk = mlp.tile([P, FFT, P], BF16, tag="w2b", bufs=2)
                    nc.sync.dma_start(w2_blk, w2[ot])
                    for oi in range(NOWN):
                        pp = psum.tile([P, 512], FP32, tag="mm", bufs=2)
                        for kk in range(FFT):
                            nc.tensor.matmul(pp, w2_blk[:, kk, :],
                                             m_sb[:, oi, kk, :],
                                             start=(kk == 0), stop=(kk == FFT - 1))
                        ost = mlp.tile([P, 512], FP32, tag="ost", bufs=2)
                        nc.vector.scalar_tensor_tensor(
                            ost, pp, b2_sb[:, ot:ot + 1], x2_sb[:, oi, ot, :],
                            ALU.add, ALU.add)
                        nc.sync.dma_start(out[:, oi, ot, :], ost)

    nc.compile()
    return nc


_NC_CACHE = None


def _get_nc():
    global _NC_CACHE
    if _NC_CACHE is None:
        _NC_CACHE = build_program(debug=bool(int(os.environ.get("KERNEL_DEBUG", "0"))))
    return _NC_CACHE


def prep_in_maps(x, ln1_g, ln1_b, ln2_g, ln2_b, Wq, bq, Wk, bk, Wv, bv,
                 Wo, bo, W1, b1, W2, b2):
    f32 = np.float32
    x = np.asarray(x, f32)
    ln1_g, ln1_b = np.asarray(ln1_g, f32), np.asarray(ln1_b, f32)
    ln2_g, ln2_b = np.asarray(ln2_g, f32), np.asarray(ln2_b, f32)
    Wq, Wk, Wv, Wo_ = (np.asarray(a, f32) for a in (Wq, Wk, Wv, Wo))
    W1, W2 = np.asarray(W1, f32), np.asarray(W2, f32)
    bq, bk, bv, bo_, b1, b2_ = (np.asarray(a, f32) for a in (bq, bk, bv, bo, b1, b2))

    # fold LN gain + mean subtraction (rank-1) into W; rstd applied to input
    def fold(g, W):
        Wg = g[:, None] * W
        return Wg - Wg.sum(0, keepdims=True) / D

    Wqg, Wkg, Wvg = fold(ln1_g, Wq), fold(ln1_g, Wk), fold(ln1_g, Wv)
    W1g = fold(ln2_g, W1)
    cq_full = ln1_b @ Wq + bq
    ck_full = ln1_b @ Wk + bk
    cv_full = ln1_b @ Wv + bv
    c1_full = ln2_b @ W1 + b1

    def wtile(W, kdim, odim):
        # [K, O] -> [O//P, P(row-within-ktile), K//P, P] block layout
        return np.ascontiguousarray(
            W.reshape(kdim // P, P, odim // P, P).transpose(2, 1, 0, 3)
        ).astype(NPBF)

    wk_t = wtile(Wkg, D, D)             # [DK, P, DK, P]
    wq_t = wtile(Wqg, D, D)             # [NP, P, DK, P] (NP == DK)
    wo_t = wtile(Wo_, D, D)
    w1_t = wtile(W1g, D, DFF)           # [FFT, P, DK, P]
    w2_t = wtile(W2, DFF, D)            # [DK, P, FFT, P]
    wv_t = np.ascontiguousarray(
        Wvg.reshape(DK, P, D).transpose(1, 0, 2)).astype(NPBF)  # [P, DK, D]

    ck_t = np.ascontiguousarray(ck_full.reshape(DK, P).T)
    cq_t = np.ascontiguousarray(cq_full.reshape(DK, P).T)
    ckq_t = np.concatenate([ck_t, cq_t], axis=1)                 # [P, 2DK]
    cvb_t = np.broadcast_to(cv_full[None, :], (P, D)).copy()
    bo_t = np.ascontiguousarray(bo_.reshape(DK, P).T)
    c1_t = np.ascontiguousarray(c1_full.reshape(FFT, P).T)
    b2_t = np.ascontiguousarray(b2_.reshape(DK, P).T)

    pp_ = np.arange(P)[:, None]
    cc_ = np.arange(512)[None, :]

    in_maps = []
    for c in range(8):
        b_idx, par = c // 2, c % 2
        slots = SLOT_TILES[par]
        arr4 = np.ascontiguousarray(x[b_idx].T).reshape(DK, P, NT, 512)
        xbf_c = np.ascontiguousarray(
            arr4[:, :, slots, :].transpose(1, 2, 0, 3)).astype(NPBF)
        xf32_c = np.ascontiguousarray(
            arr4[:, :, [slots[1], slots[3]], :].transpose(1, 2, 0, 3))
        mk = np.zeros((2, P, 8, 512), np.float32)
        for pi in range(2):
            q_tile = slots[Q_SLOTS[pi]]
            for j in range(8):
                blk = j if pi == 0 else 8 + j
                k_tile = slots[blk // 4]
                k_tok = 512 * k_tile + 128 * (blk % 4) + pp_
                q_tok = 512 * q_tile + cc_
                mk[pi, :, j, :] = (k_tok <= q_tok).astype(np.float32)
        in_maps.append({
            "xbf": xbf_c,
            "xf32": xf32_c,
            "wk": wk_t, "wv": wv_t, "wq": wq_t, "wo": wo_t,
            "w1": w1_t, "w2": w2_t,
            "masks": mk.astype(NPBF),
            "ckq": ckq_t, "cvb": cvb_t, "bo": bo_t, "c1": c1_t, "b2": b2_t,
        })
    return in_maps


def assemble_output(results):
    out = np.empty((B, T, D), np.float32)
    for c in range(8):
        b_idx, par = c // 2, c % 2
        slots = SLOT_TILES[par]
        o = results[c]["out"]  # [P, NOWN, DK, 512]
        for oi in range(NOWN):
            t0 = 512 * slots[Q_SLOTS[oi]]
            # out[b, t0+cc, 128*ot+p] = o[p, oi, ot, cc]
            out[b_idx, t0:t0 + 512, :] = (
                o[:, oi, :, :].transpose(2, 1, 0).reshape(512, D))
    return out


def kernel(**inputs):
    nc = _get_nc()
    in_maps = prep_in_maps(**inputs)
    res = run_bass_kernel_spmd(nc, in_maps, list(range(8)))
    return assemble_output(res.results)


# revision 4
# speedup vs baseline: 1.0036x; 1.0036x over previous
"""Trainium2 Bass kernel for a dense transformer block (B=4, T=2048, D=1024, H=16).

Sharding v2: 8 cores = 4 batches x 2 token-parities; no collectives.
Core (b, par) owns query tiles {0,3} (par=0) or {1,2} (par=1) of 512 tokens
(both = 5 causal units, balanced).  Every core computes LN1 + k/v projections
for all 4 key tiles of its batch (redundant within a pair), q for its own
tiles, full causal attention + complete Wo for its own tokens, then LN2 + MLP
locally.  The compiled program is identical on all cores (SPMD); per-core
differences live purely in DRAM data.

Uniformity trick: the host stages the four 512-token x tiles in per-core SLOT
order (par=0: tiles [1,0,2,3]; par=1: [0,1,3,2]) so that the two own q tiles
always sit at slots 1 and 3.  k/v are computed per slot; attention pass0
(q = slot1) runs over key blocks 0..7 (slots 0-1) and pass1 (q = slot3) over
blocks 0..15, with per-core causal masks (DRAM data) zeroing whatever a core
must not see.  Diagonal blocks (pass0: 4..7, pass1: 12..15 — the diag slot on
both cores) process only columns >= 128*j (exact shrink).

All matmul operands are bf16 (fp32 PSUM).  LayerNorm gain + mean subtraction
(rank-1) are folded into weights host-side; rstd is applied to the layer
input (xs = x * rstd) so projection epilogues are one bias-add+convert pass.
"""

import os
import sys

for _p in ("/opt/trn_rl_repo", "/root/.axon_site/_ro/trn_rl_repo"):
    if os.path.isdir(_p) and _p not in sys.path:
        sys.path.append(_p)

import ml_dtypes
import numpy as np

import concourse.bass as bass
import concourse.tile as tile
from concourse import bacc, mybir
from concourse.bass_utils import run_bass_kernel_spmd

AF = mybir.ActivationFunctionType
ALU = mybir.AluOpType
FP32 = mybir.dt.float32
BF16 = mybir.dt.bfloat16
NPBF = ml_dtypes.bfloat16

B, T, D, H = 4, 2048, 1024, 16
HD = D // H          # 64
DFF = 4 * D          # 4096
P = 128
DK = D // P          # 8   D k-tiles
NT = T // 512        # 4   512-token tiles
NP = H // 2          # 8   head pairs
FFT = DFF // P       # 32  DFF tiles
TC = T // P          # 16  128-token chunks
EPS = 1e-5
SCALE = 1.0 / 8.0    # 1/sqrt(HD)
NOWN = 2             # own 512-token tiles per core
NBLK = (8, 16)       # key blocks per attention pass
Q_SLOTS = (1, 3)     # slots holding the own q tiles

# slot -> tile permutations per parity
SLOT_TILES = ([1, 0, 2, 3], [0, 1, 3, 2])


def _shrink(pi, kt):
    if pi == 0 and kt >= 4:
        return 128 * (kt - 4)
    if pi == 1 and kt >= 12:
        return 128 * (kt - 12)
    return 0


def _masked(pi, kt):
    return kt >= 8 if pi == 1 else True


def build_program(debug=False, phases=("att", "mlp")):
    nc = bacc.Bacc("TRN2", target_bir_lowering=False, debug=False)

    # ---- DRAM I/O ----
    xbf = nc.dram_tensor("xbf", [P, NT, DK, 512], BF16, kind="ExternalInput")
    xf32 = nc.dram_tensor("xf32", [P, NOWN, DK, 512], FP32, kind="ExternalInput")
    wk = nc.dram_tensor("wk", [DK, P, DK, P], BF16, kind="ExternalInput")
    wv = nc.dram_tensor("wv", [P, DK, D], BF16, kind="ExternalInput")
    wq = nc.dram_tensor("wq", [NP, P, DK, P], BF16, kind="ExternalInput")
    wo = nc.dram_tensor("wo", [DK, P, NP, P], BF16, kind="ExternalInput")
    w1 = nc.dram_tensor("w1", [FFT, P, DK, P], BF16, kind="ExternalInput")
    w2 = nc.dram_tensor("w2", [DK, P, FFT, P], BF16, kind="ExternalInput")
    masks = nc.dram_tensor("masks", [2, P, 8, 512], BF16, kind="ExternalInput")
    ckq = nc.dram_tensor("ckq", [P, 2 * DK], FP32, kind="ExternalInput")  # [ck|cq]
    cvb = nc.dram_tensor("cvb", [P, D], FP32, kind="ExternalInput")
    bo = nc.dram_tensor("bo", [P, DK], FP32, kind="ExternalInput")
    c1 = nc.dram_tensor("c1", [P, FFT], FP32, kind="ExternalInput")
    b2 = nc.dram_tensor("b2", [P, DK], FP32, kind="ExternalInput")
    out = nc.dram_tensor("out", [P, NOWN, DK, 512], FP32, kind="ExternalOutput")
    if debug:
        dbg_kT = nc.dram_tensor("dbg_kT", [P, NP, T], BF16, kind="ExternalOutput")
        dbg_v = nc.dram_tensor("dbg_v", [P, H, TC, HD + 1], BF16,
                               kind="ExternalOutput")
        dbg_q = nc.dram_tensor("dbg_q", [P, NOWN, NP, 512], BF16,
                               kind="ExternalOutput")
        dbg_x2 = nc.dram_tensor("dbg_x2", [P, NOWN, DK, 512], BF16,
                                kind="ExternalOutput")
        dbg_y = nc.dram_tensor("dbg_y", [P, NOWN, NP, 512], BF16,
                               kind="ExternalOutput")

    with tile.TileContext(nc) as tc:
        with (
            tc.tile_pool(name="persist", bufs=1) as persist,
            tc.tile_pool(name="psum", bufs=1, space="PSUM") as psum,
        ):
            # ---- persistent small tiles ----
            ones_bf = persist.tile([P, 1], BF16)
            nc.vector.memset(ones_bf, 1.0)
            ckq_sb = persist.tile([P, 2 * DK], FP32)
            nc.sync.dma_start(ckq_sb, ckq[:, :])
            cvb_sb = persist.tile([P, D], FP32)
            nc.sync.dma_start(cvb_sb, cvb[:, :])
            bo_sb = persist.tile([P, DK], FP32)
            nc.sync.dma_start(bo_sb, bo[:, :])
            c1_sb = persist.tile([P, FFT], FP32)
            nc.sync.dma_start(c1_sb, c1[:, :])
            b2_sb = persist.tile([P, DK], FP32)
            nc.sync.dma_start(b2_sb, b2[:, :])
            eps_sb = persist.tile([1, 1], FP32)
            nc.vector.memset(eps_sb, EPS)
            # preload ACT function tables so no LoadActFuncSet lands mid-stream
            warm = persist.tile([1, 1], FP32)
            nc.scalar.activation(warm, eps_sb, AF.Sqrt)
            nc.scalar.activation(warm, eps_sb, AF.Exp)
            nc.scalar.activation(warm, eps_sb, AF.Gelu)
            row_a = persist.tile([1, 512], FP32)
            row_b = persist.tile([1, 512], FP32)
            row_mu = persist.tile([1, 512], FP32)
            x2_sb = persist.tile([P, NOWN, DK, 512], BF16)
            x2s_sb = persist.tile([P, NOWN, DK, 512], BF16)

            def ln_rows(s_ps, q_ps, rs_bf):
                nc.vector.tensor_scalar(row_mu, s_ps, 1.0 / D, None, ALU.mult)
                nc.vector.tensor_scalar(row_a, q_ps, 1.0 / D, None, ALU.mult)
                nc.vector.tensor_mul(row_b, row_mu, row_mu)
                nc.vector.tensor_sub(row_a, row_a, row_b)
                nc.scalar.activation(row_b, row_a, AF.Sqrt, bias=eps_sb)
                with nc.allow_low_precision(reason="bf16 rstd by design"):
                    nc.vector.reciprocal(rs_bf, row_b)

            with tc.tile_pool(name="work", bufs=1) as work:
              with tc.tile_pool(name="attn", bufs=1) as attn:
                wv_sb = attn.tile([P, DK, D], BF16)
                kT_sb = attn.tile([P, NP, T], BF16)        # [64*hb+e, pair, ktok]
                v_sb = attn.tile([P, H, TC, HD + 1], BF16)  # [ktok%128, h, kchunk, e|1]
                nc.vector.memset(v_sb[:, :, :, HD:HD + 1], 1.0)
                q_sb = attn.tile([P, NOWN, NP, 512], BF16)
                ysb = work.tile([P, NP, 512], BF16, tag="ysbt", bufs=1)

                def emit_den(y_ps, pt):
                    """Normalize y_ps into ysb[:, pt, :] (reciprocal of the
                    ones-column denominator, broadcast, multiply)."""
                    for hb in range(2):
                        den = work.tile([HD + 1, 512], FP32, tag="den", bufs=2)
                        nc.vector.reciprocal(den[HD:HD + 1, :],
                                             y_ps[hb][HD:HD + 1, :])
                        rec = work.tile([1, 512], FP32, tag="rec", bufs=2)
                        nc.sync.dma_start(rec, den[HD:HD + 1, :])
                        rb = work.tile([HD, 512], FP32, tag="rb", bufs=2)
                        nc.gpsimd.partition_broadcast(rb, rec)
                        if hb == 0:
                            nc.vector.tensor_mul(ysb[0:HD, pt, :],
                                                 y_ps[hb][0:HD, :], rb)
                        else:
                            yst = work.tile([HD, 512], BF16, tag="yst", bufs=1)
                            nc.vector.tensor_mul(yst, y_ps[hb][0:HD, :], rb)
                            nc.sync.dma_start(ysb[HD:2 * HD, pt, :], yst)

                def attention(pi, fillers=()):
                    fillers = list(fillers)
                    nblk = NBLK[pi]
                    mask_sb = work.tile([P, 8, 512], BF16, tag="mask", bufs=1)
                    nc.sync.dma_start(mask_sb, masks[pi])
                    prev_y = None
                    for pt in range(NP):
                        for _ in range(2):
                            if fillers:
                                fillers.pop(0)()
                        y_ps = [psum.tile([HD + 1, 512], FP32, name=f"yps{hb}",
                                          tag="y", bufs=2) for hb in range(2)]
                        for kt in range(nblk):
                            off = _shrink(pi, kt)
                            pexp = []
                            for hb in range(2):
                                hsl = slice(hb * HD, (hb + 1) * HD)
                                s_ps = psum.tile([P, 512], FP32, tag="mm", bufs=2)
                                nc.tensor.matmul(
                                    s_ps[:, off:],
                                    kT_sb[hsl, pt, kt * P:(kt + 1) * P],
                                    q_sb[hsl, pi, pt, off:], start=True, stop=True)
                                pe = work.tile([P, 512], BF16, tag="pexp", bufs=3)
                                nc.scalar.activation(pe[:, off:], s_ps[:, off:],
                                                     AF.Exp, scale=SCALE)
                                if _masked(pi, kt):
                                    mi = kt if pi == 0 else kt - 8
                                    nc.vector.tensor_mul(pe[:, off:], pe[:, off:],
                                                         mask_sb[:, mi, off:])
                                pexp.append(pe)
                            # normalize the previous pair's y while this one's
                            # exp/AV chain runs (keeps the DVE den chain out of
                            # the critical path)
                            if kt == 1 and prev_y is not None:
                                emit_den(prev_y, pt - 1)
                            for hb in range(2):
                                nc.tensor.matmul(
                                    y_ps[hb][:, off:], v_sb[:, 2 * pt + hb, kt, :],
                                    pexp[hb][:, off:],
                                    start=(kt == 0), stop=(kt == nblk - 1))
                        prev_y = y_ps
                    emit_den(prev_y, NP - 1)
                    if debug:
                        nc.sync.dma_start(dbg_y[:, pi], ysb)
                    # ---- Wo + x2 + LN2 stats ----
                    s2_ps = psum.tile([1, 512], FP32, tag="st", bufs=2)
                    q2_ps = psum.tile([1, 512], FP32, tag="st", bufs=2)
                    for ot in range(DK):
                        wo_blk = work.tile([P, NP, P], BF16, tag="wob", bufs=2)
                        nc.sync.dma_start(wo_blk, wo[ot])
                        xo_blk = work.tile([P, 512], FP32, tag="xob", bufs=1)
                        nc.sync.dma_start(xo_blk, xf32[:, pi, ot, :])
                        pp = psum.tile([P, 512], FP32, tag="mm", bufs=2)
                        for pt in range(NP):
                            nc.tensor.matmul(pp, wo_blk[:, pt, :], ysb[:, pt, :],
                                             start=(pt == 0), stop=(pt == NP - 1))
                        nc.vector.scalar_tensor_tensor(
                            x2_sb[:, pi, ot, :], pp, bo_sb[:, ot:ot + 1], xo_blk,
                            ALU.add, ALU.add)
                        nc.tensor.matmul(s2_ps, ones_bf, x2_sb[:, pi, ot, :],
                                         start=(ot == 0), stop=(ot == DK - 1))
                        xsq = work.tile([P, 512], BF16, tag="xsq", bufs=1)
                        nc.vector.tensor_mul(xsq, x2_sb[:, pi, ot, :],
                                             x2_sb[:, pi, ot, :])
                        nc.tensor.matmul(q2_ps, ones_bf, xsq,
                                         start=(ot == 0), stop=(ot == DK - 1))
                    rs2_bf = work.tile([1, 512], BF16, tag="rsrow", bufs=2)
                    ln_rows(s2_ps, q2_ps, rs2_bf)
                    rsb2 = work.tile([P, 512], BF16, tag="rsb", bufs=2)
                    nc.gpsimd.partition_broadcast(rsb2, rs2_bf)
                    for kt in range(DK):
                        nc.vector.tensor_mul(x2s_sb[:, pi, kt, :],
                                             x2_sb[:, pi, kt, :], rsb2)

                # ==== per 512-token slot: LN1 stats, k/v (+q); attention ====
                def load_and_stats(tt):
                    """DMA x tile, LN1 stats, then scale it in place by rstd."""
                    xt = work.tile([P, DK, 512], BF16, tag="xt", bufs=2)
                    nc.sync.dma_start(xt, xbf[:, tt])
                    s_ps = psum.tile([1, 512], FP32, tag="st", bufs=2)
                    q_ps = psum.tile([1, 512], FP32, tag="st", bufs=2)
                    for kt in range(DK):
                        nc.tensor.matmul(s_ps, ones_bf, xt[:, kt, :],
                                         start=(kt == 0), stop=(kt == DK - 1))
                    for kt in range(DK):
                        xsq = work.tile([P, 512], BF16, tag="xsq", bufs=1)
                        nc.vector.tensor_mul(xsq, xt[:, kt, :], xt[:, kt, :])
                        nc.tensor.matmul(q_ps, ones_bf, xsq,
                                         start=(kt == 0), stop=(kt == DK - 1))
                    rs_bf = work.tile([1, 512], BF16, tag="rsrow", bufs=2)
                    ln_rows(s_ps, q_ps, rs_bf)
                    rsb = work.tile([P, 512], BF16, tag="rsb", bufs=2)
                    nc.gpsimd.partition_broadcast(rsb, rs_bf)
                    for kt in range(DK):
                        nc.vector.tensor_mul(xt[:, kt, :], xt[:, kt, :], rsb)
                    return xt

                def kproj_unit(xs, tt, ot):
                    wk_blk = work.tile([P, DK, P], BF16, tag="wkb", bufs=2)
                    nc.sync.dma_start(wk_blk, wk[ot])
                    pp = psum.tile([P, 512], FP32, tag="mm", bufs=2)
                    for kt in range(DK):
                        nc.tensor.matmul(pp, wk_blk[:, kt, :], xs[:, kt, :],
                                         start=(kt == 0), stop=(kt == DK - 1))
                    nc.vector.tensor_scalar(
                        kT_sb[:, ot, tt * 512:(tt + 1) * 512], pp,
                        ckq_sb[:, ot:ot + 1], None, ALU.add)

                def vproj_unit(xs, tt, st):
                    # halves sequential: holds only one psum slot at a time so
                    # interleaved attention scores keep their ping-pong buffer
                    for half in range(2):
                        pv = psum.tile([P, 512], FP32, tag="mm", bufs=2)
                        for kt in range(DK):
                            nc.tensor.matmul(
                                pv, xs[:, kt, st * P:(st + 1) * P],
                                wv_sb[:, kt, half * 512:(half + 1) * 512],
                                start=(kt == 0), stop=(kt == DK - 1))
                        dest = v_sb[:, half * NP:(half + 1) * NP, tt * 4 + st, 0:HD]
                        nc.vector.tensor_tensor(
                            dest,
                            pv.rearrange("p (h e) -> p h e", h=NP),
                            cvb_sb[:, half * 512:(half + 1) * 512]
                            .rearrange("p (h e) -> p h e", h=NP), ALU.add)

                def qproj_unit(xs, oi, ot):
                    wq_blk = work.tile([P, DK, P], BF16, tag="wqb", bufs=2)
                    nc.sync.dma_start(wq_blk, wq[ot])
                    pp = psum.tile([P, 512], FP32, tag="mm", bufs=2)
                    for kt in range(DK):
                        nc.tensor.matmul(pp, wq_blk[:, kt, :], xs[:, kt, :],
                                         start=(kt == 0), stop=(kt == DK - 1))
                    nc.vector.tensor_scalar(
                        q_sb[:, oi, ot, :], pp,
                        ckq_sb[:, DK + ot:DK + ot + 1], None, ALU.add)

                xs0 = load_and_stats(0)
                nc.sync.dma_start(wv_sb, wv[:, :, :])
                for ot in range(DK):
                    kproj_unit(xs0, 0, ot)
                xs1 = load_and_stats(1)
                for st in range(4):
                    vproj_unit(xs0, 0, st)
                for ot in range(DK):
                    kproj_unit(xs1, 1, ot)
                xs2 = load_and_stats(2)
                for st in range(4):
                    vproj_unit(xs1, 1, st)
                for ot in range(NP):
                    qproj_unit(xs1, 0, ot)
                # attention pass0 interleaved with slot2 projections + slot3
                # stats (fills the PE during the ACT-bound exp stretches)
                holder = {}
                fillers = (
                    [lambda ot=ot: kproj_unit(xs2, 2, ot) for ot in range(DK)]
                    + [lambda: holder.__setitem__("xs3", load_and_stats(3))]
                    + [lambda st=st: vproj_unit(xs2, 2, st) for st in range(4)]
                )
                if "att" in phases:
                    attention(0, fillers)
                else:
                    for f in fillers:
                        f()
                xs3 = holder["xs3"]
                for ot in range(DK):
                    kproj_unit(xs3, 3, ot)
                for st in range(4):
                    vproj_unit(xs3, 3, st)
                for ot in range(NP):
                    qproj_unit(xs3, 1, ot)
                if "att" in phases:
                    attention(1)

                if debug:
                    nc.sync.dma_start(dbg_kT[:, :, :], kT_sb)
                    nc.sync.dma_start(dbg_v[:, :, :, :], v_sb)
                    nc.sync.dma_start(dbg_q[:, :, :, :], q_sb)
                    nc.sync.dma_start(dbg_x2[:, :, :, :], x2_sb)

              # ==== MLP over both own tiles (reuses the attn pool space;
              # fc1 for own-tile 0 overlaps the Wo/LN2 tail of pass 1) ====
              with tc.tile_pool(name="mlp", bufs=1) as mlp:
                if "mlp" in phases:
                  m_sb = mlp.tile([P, NOWN, FFT, 512], BF16)
                  for oi in range(NOWN):
                    for fft in range(FFT):
                        w1_blk = mlp.tile([P, DK, P], BF16, tag="w1b", bufs=3)
                        nc.sync.dma_start(w1_blk, w1[fft])
                        pp = psum.tile([P, 512], FP32, tag="mm", bufs=2)
                        for kt in range(DK):
                            nc.tensor.matmul(pp, w1_blk[:, kt, :],
                                             x2s_sb[:, oi, kt, :],
                                             start=(kt == 0), stop=(kt == DK - 1))
                        nc.scalar.activation(m_sb[:, oi, fft, :], pp, AF.Gelu,
                                             bias=c1_sb[:, fft:fft + 1])
                  for ot in range(DK):
                    w2_blk = mlp.tile([P, FFT, P], BF16, tag="w2b", bufs=2)
                    nc.sync.dma_start(w2_blk, w2[ot])
                    for oi in range(NOWN):
                        pp = psum.tile([P, 512], FP32, tag="mm", bufs=2)
                        for kk in range(FFT):
                            nc.tensor.matmul(pp, w2_blk[:, kk, :],
                                             m_sb[:, oi, kk, :],
                                             start=(kk == 0), stop=(kk == FFT - 1))
                        ost = mlp.tile([P, 512], FP32, tag="ost", bufs=2)
                        nc.vector.scalar_tensor_tensor(
                            ost, pp, b2_sb[:, ot:ot + 1], x2_sb[:, oi, ot, :],
                            ALU.add, ALU.add)
                        nc.sync.dma_start(out[:, oi, ot, :], ost)

    nc.compile()
    return nc


_NC_CACHE = None


def _get_nc():
    global _NC_CACHE
    if _NC_CACHE is None:
        _NC_CACHE = build_program(debug=bool(int(os.environ.get("KERNEL_DEBUG", "0"))))
    return _NC_CACHE


def prep_in_maps(x, ln1_g, ln1_b, ln2_g, ln2_b, Wq, bq, Wk, bk, Wv, bv,
                 Wo, bo, W1, b1, W2, b2):
    f32 = np.float32
    x = np.asarray(x, f32)
    ln1_g, ln1_b = np.asarray(ln1_g, f32), np.asarray(ln1_b, f32)
    ln2_g, ln2_b = np.asarray(ln2_g, f32), np.asarray(ln2_b, f32)
    Wq, Wk, Wv, Wo_ = (np.asarray(a, f32) for a in (Wq, Wk, Wv, Wo))
    W1, W2 = np.asarray(W1, f32), np.asarray(W2, f32)
    bq, bk, bv, bo_, b1, b2_ = (np.asarray(a, f32) for a in (bq, bk, bv, bo, b1, b2))

    # fold LN gain + mean subtraction (rank-1) into W; rstd applied to input
    def fold(g, W):
        Wg = g[:, None] * W
        return Wg - Wg.sum(0, keepdims=True) / D

    Wqg, Wkg, Wvg = fold(ln1_g, Wq), fold(ln1_g, Wk), fold(ln1_g, Wv)
    W1g = fold(ln2_g, W1)
    cq_full = ln1_b @ Wq + bq
    ck_full = ln1_b @ Wk + bk
    cv_full = ln1_b @ Wv + bv
    c1_full = ln2_b @ W1 + b1

    def wtile(W, kdim, odim):
        # [K, O] -> [O//P, P(row-within-ktile), K//P, P] block layout
        return np.ascontiguousarray(
            W.reshape(kdim // P, P, odim // P, P).transpose(2, 1, 0, 3)
        ).astype(NPBF)

    wk_t = wtile(Wkg, D, D)             # [DK, P, DK, P]
    wq_t = wtile(Wqg, D, D)             # [NP, P, DK, P] (NP == DK)
    wo_t = wtile(Wo_, D, D)
    w1_t = wtile(W1g, D, DFF)           # [FFT, P, DK, P]
    w2_t = wtile(W2, DFF, D)            # [DK, P, FFT, P]
    wv_t = np.ascontiguousarray(
        Wvg.reshape(DK, P, D).transpose(1, 0, 2)).astype(NPBF)  # [P, DK, D]

    ck_t = np.ascontiguousarray(ck_full.reshape(DK, P).T)
    cq_t = np.ascontiguousarray(cq_full.reshape(DK, P).T)
    ckq_t = np.concatenate([ck_t, cq_t], axis=1)                 # [P, 2DK]
    cvb_t = np.broadcast_to(cv_full[None, :], (P, D)).copy()
    bo_t = np.ascontiguousarray(bo_.reshape(DK, P).T)
    c1_t = np.ascontiguousarray(c1_full.reshape(FFT, P).T)
    b2_t = np.ascontiguousarray(b2_.reshape(DK, P).T)

    pp_ = np.arange(P)[:, None]
    cc_ = np.arange(512)[None, :]

    in_maps = []
    for c in range(8):
        b_idx, par = c // 2, c % 2
        slots = SLOT_TILES[par]
        arr4 = np.ascontiguousarray(x[b_idx].T).reshape(DK, P, NT, 512)
        xbf_c = np.ascontiguousarray(
            arr4[:, :, slots, :].transpose(1, 2, 0, 3)).astype(NPBF)
        xf32_c = np.ascontiguousarray(
            arr4[:, :, [slots[1], slots[3]], :].transpose(1, 2, 0, 3))
        mk = np.zeros((2, P, 8, 512), np.float32)
        for pi in range(2):
            q_tile = slots[Q_SLOTS[pi]]
            for j in range(8):
                blk = j if pi == 0 else 8 + j
                k_tile = slots[blk // 4]
                k_tok = 512 * k_tile + 128 * (blk % 4) + pp_
                q_tok = 512 * q_tile + cc_
                mk[pi, :, j, :] = (k_tok <= q_tok).astype(np.float32)
        in_maps.append({
            "xbf": xbf_c,
            "xf32": xf32_c,
            "wk": wk_t, "wv": wv_t, "wq": wq_t, "wo": wo_t,
            "w1": w1_t, "w2": w2_t,
            "masks": mk.astype(NPBF),
            "ckq": ckq_t, "cvb": cvb_t, "bo": bo_t, "c1": c1_t, "b2": b2_t,
        })
    return in_maps


def assemble_output(results):
    out = np.empty((B, T, D), np.float32)
    for c in range(8):
        b_idx, par = c // 2, c % 2
        slots = SLOT_TILES[par]
        o = results[c]["out"]  # [P, NOWN, DK, 512]
        for oi in range(NOWN):
            t0 = 512 * slots[Q_SLOTS[oi]]
            # out[b, t0+cc, 128*ot+p] = o[p, oi, ot, cc]
            out[b_idx, t0:t0 + 512, :] = (
                o[:, oi, :, :].transpose(2, 1, 0).reshape(512, D))
    return out


def kernel(**inputs):
    nc = _get_nc()
    in_maps = prep_in_maps(**inputs)
    res = run_bass_kernel_spmd(nc, in_maps, list(range(8)))
    return assemble_output(res.results)


# revision 5
# speedup vs baseline: 1.2277x; 1.2233x over previous
"""Trainium2 Bass kernel for a dense transformer block (B=4, T=2048, D=1024, H=16).

Sharding v2: 8 cores = 4 batches x 2 token-parities; no collectives.
Core (b, par) owns query tiles {0,3} (par=0) or {1,2} (par=1) of 512 tokens
(both = 5 causal units, balanced).  Every core computes LN1 + k/v projections
for all 4 key tiles of its batch (redundant within a pair), q for its own
tiles, full causal attention + complete Wo for its own tokens, then LN2 + MLP
locally.  The compiled program is identical on all cores (SPMD); per-core
differences live purely in DRAM data.

Uniformity trick: the host stages the four 512-token x tiles in per-core SLOT
order (par=0: tiles [1,0,2,3]; par=1: [0,1,3,2]) so that the two own q tiles
always sit at slots 1 and 3.  k/v are computed per slot; attention pass0
(q = slot1) runs over key blocks 0..7 (slots 0-1) and pass1 (q = slot3) over
blocks 0..15, with per-core causal masks (DRAM data) zeroing whatever a core
must not see.  Diagonal blocks (pass0: 4..7, pass1: 12..15 — the diag slot on
both cores) process only columns >= 128*j (exact shrink).

All matmul operands are bf16 (fp32 PSUM).  LayerNorm gain + mean subtraction
(rank-1) are folded into weights host-side; rstd is applied to the layer
input (xs = x * rstd) so projection epilogues are one bias-add+convert pass.
"""

import os
import sys

for _p in ("/opt/trn_rl_repo", "/root/.axon_site/_ro/trn_rl_repo"):
    if os.path.isdir(_p) and _p not in sys.path:
        sys.path.append(_p)

import ml_dtypes
import numpy as np

import concourse.bass as bass
import concourse.tile as tile
from concourse import bacc, mybir
from concourse.bass_utils import run_bass_kernel_spmd

AF = mybir.ActivationFunctionType
ALU = mybir.AluOpType
FP32 = mybir.dt.float32
BF16 = mybir.dt.bfloat16
NPBF = ml_dtypes.bfloat16

B, T, D, H = 4, 2048, 1024, 16
HD = D // H          # 64
DFF = 4 * D          # 4096
P = 128
DK = D // P          # 8   D k-tiles
NT = T // 512        # 4   512-token tiles
NP = H // 2          # 8   head pairs
FFT = DFF // P       # 32  DFF tiles
TC = T // P          # 16  128-token chunks
EPS = 1e-5
SCALE = 1.0 / 8.0    # 1/sqrt(HD)
NOWN = 2             # own 512-token tiles per core
NBLK = (8, 16)       # key blocks per attention pass
Q_SLOTS = (1, 3)     # slots holding the own q tiles

# slot -> tile permutations per parity
SLOT_TILES = ([1, 0, 2, 3], [0, 1, 3, 2])


def _shrink(pi, kt):
    if pi == 0 and kt >= 4:
        return 128 * (kt - 4)
    if pi == 1 and kt >= 12:
        return 128 * (kt - 12)
    return 0


def _masked(pi, kt):
    return kt >= 8 if pi == 1 else True


def build_program(debug=False, phases=("att", "mlp")):
    nc = bacc.Bacc("TRN2", target_bir_lowering=False, debug=False)

    # ---- DRAM I/O ----
    xbf = nc.dram_tensor("xbf", [P, NT, DK, 512], BF16, kind="ExternalInput")
    xf32 = nc.dram_tensor("xf32", [P, NOWN, DK, 512], FP32, kind="ExternalInput")
    wk = nc.dram_tensor("wk", [DK, P, DK, P], BF16, kind="ExternalInput")
    wv = nc.dram_tensor("wv", [P, DK, D], BF16, kind="ExternalInput")
    wq = nc.dram_tensor("wq", [NP, P, DK, P], BF16, kind="ExternalInput")
    wo = nc.dram_tensor("wo", [DK, P, NP, P], BF16, kind="ExternalInput")
    w1 = nc.dram_tensor("w1", [FFT, P, DK, P], BF16, kind="ExternalInput")
    w2 = nc.dram_tensor("w2", [DK, P, FFT, P], BF16, kind="ExternalInput")
    masks = nc.dram_tensor("masks", [2, P, 8, 512], BF16, kind="ExternalInput")
    ckq = nc.dram_tensor("ckq", [P, 2 * DK], FP32, kind="ExternalInput")  # [ck|cq]
    cvb = nc.dram_tensor("cvb", [P, D], FP32, kind="ExternalInput")
    bo = nc.dram_tensor("bo", [P, DK], FP32, kind="ExternalInput")
    c1 = nc.dram_tensor("c1", [P, FFT], FP32, kind="ExternalInput")
    b2 = nc.dram_tensor("b2", [P, DK], FP32, kind="ExternalInput")
    out = nc.dram_tensor("out", [P, NOWN, DK, 512], FP32, kind="ExternalOutput")
    if debug:
        dbg_kT = nc.dram_tensor("dbg_kT", [P, NP, T], BF16, kind="ExternalOutput")
        dbg_v = nc.dram_tensor("dbg_v", [P, H, TC, HD + 1], BF16,
                               kind="ExternalOutput")
        dbg_q = nc.dram_tensor("dbg_q", [P, NOWN, NP, 512], BF16,
                               kind="ExternalOutput")
        dbg_x2 = nc.dram_tensor("dbg_x2", [P, NOWN, DK, 512], BF16,
                                kind="ExternalOutput")
        dbg_y = nc.dram_tensor("dbg_y", [P, NOWN, NP, 512], BF16,
                               kind="ExternalOutput")

    with tile.TileContext(nc) as tc:
        with (
            tc.tile_pool(name="persist", bufs=1) as persist,
            tc.tile_pool(name="psum", bufs=1, space="PSUM") as psum,
        ):
            # ---- persistent small tiles ----
            ones_bf = persist.tile([P, 1], BF16)
            nc.vector.memset(ones_bf, 1.0)
            ckq_sb = persist.tile([P, 2 * DK], FP32)
            nc.sync.dma_start(ckq_sb, ckq[:, :])
            cvb_sb = persist.tile([P, D], FP32)
            nc.sync.dma_start(cvb_sb, cvb[:, :])
            bo_sb = persist.tile([P, DK], FP32)
            nc.sync.dma_start(bo_sb, bo[:, :])
            c1_sb = persist.tile([P, FFT], FP32)
            nc.sync.dma_start(c1_sb, c1[:, :])
            b2_sb = persist.tile([P, DK], FP32)
            nc.sync.dma_start(b2_sb, b2[:, :])
            eps_sb = persist.tile([1, 1], FP32)
            nc.vector.memset(eps_sb, EPS)
            # preload ACT function tables so no LoadActFuncSet lands mid-stream
            warm = persist.tile([1, 1], FP32)
            nc.scalar.activation(warm, eps_sb, AF.Sqrt)
            nc.scalar.activation(warm, eps_sb, AF.Exp)
            nc.scalar.activation(warm, eps_sb, AF.Gelu)
            row_a = persist.tile([1, 512], FP32)
            row_b = persist.tile([1, 512], FP32)
            row_mu = persist.tile([1, 512], FP32)
            x2_sb = persist.tile([P, NOWN, DK, 512], BF16)
            x2s_sb = persist.tile([P, NOWN, DK, 512], BF16)

            def ln_rows(s_ps, q_ps, rs_bf):
                nc.vector.tensor_scalar(row_mu, s_ps, 1.0 / D, None, ALU.mult)
                nc.vector.tensor_scalar(row_a, q_ps, 1.0 / D, None, ALU.mult)
                nc.vector.tensor_mul(row_b, row_mu, row_mu)
                nc.vector.tensor_sub(row_a, row_a, row_b)
                nc.scalar.activation(row_b, row_a, AF.Sqrt, bias=eps_sb)
                with nc.allow_low_precision(reason="bf16 rstd by design"):
                    nc.vector.reciprocal(rs_bf, row_b)

            with tc.tile_pool(name="work", bufs=1) as work:
              with tc.tile_pool(name="attn", bufs=1) as attn:
                wv_sb = attn.tile([P, DK, D], BF16)
                kT_sb = attn.tile([P, NP, T], BF16)        # [64*hb+e, pair, ktok]
                v_sb = attn.tile([P, H, TC, HD + 1], BF16)  # [ktok%128, h, kchunk, e|1]
                nc.vector.memset(v_sb[:, :, :, HD:HD + 1], 1.0)
                q_sb = attn.tile([P, NOWN, NP, 512], BF16)
                ysb = work.tile([P, NP, 512], BF16, tag="ysbt", bufs=1)

                def emit_den(y_ps, pt):
                    """Normalize y_ps into ysb[:, pt, :] (reciprocal of the
                    ones-column denominator, broadcast, multiply)."""
                    for hb in range(2):
                        den = work.tile([HD + 1, 512], FP32, tag="den", bufs=2)
                        nc.vector.reciprocal(den[HD:HD + 1, :],
                                             y_ps[hb][HD:HD + 1, :])
                        rec = work.tile([1, 512], FP32, tag="rec", bufs=2)
                        nc.sync.dma_start(rec, den[HD:HD + 1, :])
                        rb = work.tile([HD, 512], FP32, tag="rb", bufs=2)
                        nc.gpsimd.partition_broadcast(rb, rec)
                        if hb == 0:
                            nc.vector.tensor_mul(ysb[0:HD, pt, :],
                                                 y_ps[hb][0:HD, :], rb)
                        else:
                            yst = work.tile([HD, 512], BF16, tag="yst", bufs=1)
                            nc.vector.tensor_mul(yst, y_ps[hb][0:HD, :], rb)
                            nc.sync.dma_start(ysb[HD:2 * HD, pt, :], yst)

                def attention(pi, fillers=(), pops=2):
                    fillers = list(fillers)
                    nblk = NBLK[pi]
                    mask_sb = work.tile([P, 8, 512], BF16, tag="mask", bufs=1)
                    nc.sync.dma_start(mask_sb, masks[pi])
                    prev_y = None
                    for pt in range(NP):
                        for _ in range(pops):
                            if fillers:
                                fillers.pop(0)()
                        y_ps = [psum.tile([HD + 1, 512], FP32, name=f"yps{hb}",
                                          tag="y", bufs=2) for hb in range(2)]
                        for kt in range(nblk):
                            off = _shrink(pi, kt)
                            pexp = []
                            for hb in range(2):
                                hsl = slice(hb * HD, (hb + 1) * HD)
                                s_ps = psum.tile([P, 512], FP32, tag="mm", bufs=2)
                                nc.tensor.matmul(
                                    s_ps[:, off:],
                                    kT_sb[hsl, pt, kt * P:(kt + 1) * P],
                                    q_sb[hsl, pi, pt, off:], start=True, stop=True)
                                pe = work.tile([P, 512], BF16, tag="pexp", bufs=3)
                                nc.scalar.activation(pe[:, off:], s_ps[:, off:],
                                                     AF.Exp, scale=SCALE)
                                if _masked(pi, kt):
                                    mi = kt if pi == 0 else kt - 8
                                    nc.vector.tensor_mul(pe[:, off:], pe[:, off:],
                                                         mask_sb[:, mi, off:])
                                pexp.append(pe)
                            # normalize the previous pair's y while this one's
                            # exp/AV chain runs (keeps the DVE den chain out of
                            # the critical path)
                            if kt == 1 and prev_y is not None:
                                emit_den(prev_y, pt - 1)
                            for hb in range(2):
                                nc.tensor.matmul(
                                    y_ps[hb][:, off:], v_sb[:, 2 * pt + hb, kt, :],
                                    pexp[hb][:, off:],
                                    start=(kt == 0), stop=(kt == nblk - 1))
                        prev_y = y_ps
                    emit_den(prev_y, NP - 1)
                    if debug:
                        nc.sync.dma_start(dbg_y[:, pi], ysb)
                    # ---- Wo + x2 + LN2 stats ----
                    s2_ps = psum.tile([1, 512], FP32, tag="st", bufs=2)
                    q2_ps = psum.tile([1, 512], FP32, tag="st", bufs=2)
                    for ot in range(DK):
                        wo_blk = work.tile([P, NP, P], BF16, tag="wob", bufs=2)
                        nc.sync.dma_start(wo_blk, wo[ot])
                        xo_blk = work.tile([P, 512], FP32, tag="xob", bufs=1)
                        nc.sync.dma_start(xo_blk, xf32[:, pi, ot, :])
                        pp = psum.tile([P, 512], FP32, tag="mm", bufs=2)
                        for pt in range(NP):
                            nc.tensor.matmul(pp, wo_blk[:, pt, :], ysb[:, pt, :],
                                             start=(pt == 0), stop=(pt == NP - 1))
                        nc.vector.scalar_tensor_tensor(
                            x2_sb[:, pi, ot, :], pp, bo_sb[:, ot:ot + 1], xo_blk,
                            ALU.add, ALU.add)
                        nc.tensor.matmul(s2_ps, ones_bf, x2_sb[:, pi, ot, :],
                                         start=(ot == 0), stop=(ot == DK - 1))
                        xsq = work.tile([P, 512], BF16, tag="xsq", bufs=1)
                        nc.vector.tensor_mul(xsq, x2_sb[:, pi, ot, :],
                                             x2_sb[:, pi, ot, :])
                        nc.tensor.matmul(q2_ps, ones_bf, xsq,
                                         start=(ot == 0), stop=(ot == DK - 1))
                    rs2_bf = work.tile([1, 512], BF16, tag="rsrow", bufs=2)
                    ln_rows(s2_ps, q2_ps, rs2_bf)
                    rsb2 = work.tile([P, 512], BF16, tag="rsb", bufs=2)
                    nc.gpsimd.partition_broadcast(rsb2, rs2_bf)
                    for kt in range(DK):
                        nc.vector.tensor_mul(x2s_sb[:, pi, kt, :],
                                             x2_sb[:, pi, kt, :], rsb2)

                # ==== per 512-token slot: LN1 stats, k/v (+q); attention ====
                def load_and_stats(tt):
                    """DMA x tile, LN1 stats, then scale it in place by rstd."""
                    xt = work.tile([P, DK, 512], BF16, tag="xt", bufs=2)
                    nc.sync.dma_start(xt, xbf[:, tt])
                    s_ps = psum.tile([1, 512], FP32, tag="st", bufs=2)
                    q_ps = psum.tile([1, 512], FP32, tag="st", bufs=2)
                    for kt in range(DK):
                        nc.tensor.matmul(s_ps, ones_bf, xt[:, kt, :],
                                         start=(kt == 0), stop=(kt == DK - 1))
                    for kt in range(DK):
                        xsq = work.tile([P, 512], BF16, tag="xsq", bufs=1)
                        nc.vector.tensor_mul(xsq, xt[:, kt, :], xt[:, kt, :])
                        nc.tensor.matmul(q_ps, ones_bf, xsq,
                                         start=(kt == 0), stop=(kt == DK - 1))
                    rs_bf = work.tile([1, 512], BF16, tag="rsrow", bufs=2)
                    ln_rows(s_ps, q_ps, rs_bf)
                    rsb = work.tile([P, 512], BF16, tag="rsb", bufs=2)
                    nc.gpsimd.partition_broadcast(rsb, rs_bf)
                    for kt in range(DK):
                        nc.vector.tensor_mul(xt[:, kt, :], xt[:, kt, :], rsb)
                    return xt

                def kproj_unit(xs, tt, ot):
                    wk_blk = work.tile([P, DK, P], BF16, tag="wkb", bufs=2)
                    nc.sync.dma_start(wk_blk, wk[ot])
                    pp = psum.tile([P, 512], FP32, tag="mm", bufs=2)
                    for kt in range(DK):
                        nc.tensor.matmul(pp, wk_blk[:, kt, :], xs[:, kt, :],
                                         start=(kt == 0), stop=(kt == DK - 1))
                    nc.vector.tensor_scalar(
                        kT_sb[:, ot, tt * 512:(tt + 1) * 512], pp,
                        ckq_sb[:, ot:ot + 1], None, ALU.add)

                def vproj_unit(xs, tt, st):
                    # halves sequential: holds only one psum slot at a time so
                    # interleaved attention scores keep their ping-pong buffer
                    for half in range(2):
                        pv = psum.tile([P, 512], FP32, tag="mm", bufs=2)
                        for kt in range(DK):
                            nc.tensor.matmul(
                                pv, xs[:, kt, st * P:(st + 1) * P],
                                wv_sb[:, kt, half * 512:(half + 1) * 512],
                                start=(kt == 0), stop=(kt == DK - 1))
                        dest = v_sb[:, half * NP:(half + 1) * NP, tt * 4 + st, 0:HD]
                        nc.vector.tensor_tensor(
                            dest,
                            pv.rearrange("p (h e) -> p h e", h=NP),
                            cvb_sb[:, half * 512:(half + 1) * 512]
                            .rearrange("p (h e) -> p h e", h=NP), ALU.add)

                def qproj_unit(xs, oi, ot):
                    wq_blk = work.tile([P, DK, P], BF16, tag="wqb", bufs=2)
                    nc.sync.dma_start(wq_blk, wq[ot])
                    pp = psum.tile([P, 512], FP32, tag="mm", bufs=2)
                    for kt in range(DK):
                        nc.tensor.matmul(pp, wq_blk[:, kt, :], xs[:, kt, :],
                                         start=(kt == 0), stop=(kt == DK - 1))
                    nc.vector.tensor_scalar(
                        q_sb[:, oi, ot, :], pp,
                        ckq_sb[:, DK + ot:DK + ot + 1], None, ALU.add)

                xs0 = load_and_stats(0)
                nc.sync.dma_start(wv_sb, wv[:, :, :])
                for ot in range(DK):
                    kproj_unit(xs0, 0, ot)
                xs1 = load_and_stats(1)
                for st in range(4):
                    vproj_unit(xs0, 0, st)
                for ot in range(DK):
                    kproj_unit(xs1, 1, ot)
                xs2 = load_and_stats(2)
                for st in range(4):
                    vproj_unit(xs1, 1, st)
                for ot in range(NP):
                    qproj_unit(xs1, 0, ot)
                # attention pass0 interleaved with slot2 projections + slot3
                # stats (fills the PE during the ACT-bound exp stretches)
                holder = {}
                fillers = (
                    [lambda ot=ot: kproj_unit(xs2, 2, ot) for ot in range(DK)]
                    + [lambda: holder.__setitem__("xs3", load_and_stats(3))]
                    + [lambda st=st: vproj_unit(xs2, 2, st) for st in range(4)]
                )
                if "att" in phases:
                    attention(0, fillers)
                else:
                    for f in fillers:
                        f()
                xs3 = holder["xs3"]
                for st in range(4):
                    vproj_unit(xs3, 3, st)
                for ot in range(NP):
                    qproj_unit(xs3, 1, ot)
                fillers1 = [lambda ot=ot: kproj_unit(xs3, 3, ot)
                            for ot in range(DK)]
                if "att" in phases:
                    attention(1, fillers1, pops=1)
                else:
                    for f in fillers1:
                        f()

                if debug:
                    nc.sync.dma_start(dbg_kT[:, :, :], kT_sb)
                    nc.sync.dma_start(dbg_v[:, :, :, :], v_sb)
                    nc.sync.dma_start(dbg_q[:, :, :, :], q_sb)
                    nc.sync.dma_start(dbg_x2[:, :, :, :], x2_sb)

              # ==== MLP over both own tiles (reuses the attn pool space;
              # fc1 for own-tile 0 overlaps the Wo/LN2 tail of pass 1) ====
              with tc.tile_pool(name="mlp", bufs=1) as mlp:
                if "mlp" in phases:
                  m_sb = mlp.tile([P, NOWN, FFT, 512], BF16)
                  for oi in range(NOWN):
                    for fft in range(FFT):
                        w1_blk = mlp.tile([P, DK, P], BF16, tag="w1b", bufs=3)
                        nc.sync.dma_start(w1_blk, w1[fft])
                        pp = psum.tile([P, 512], FP32, tag="mm", bufs=2)
                        for kt in range(DK):
                            nc.tensor.matmul(pp, w1_blk[:, kt, :],
                                             x2s_sb[:, oi, kt, :],
                                             start=(kt == 0), stop=(kt == DK - 1))
                        nc.scalar.activation(m_sb[:, oi, fft, :], pp, AF.Gelu,
                                             bias=c1_sb[:, fft:fft + 1])
                  for ot in range(DK):
                    w2_blk = mlp.tile([P, FFT, P], BF16, tag="w2b", bufs=2)
                    nc.sync.dma_start(w2_blk, w2[ot])
                    for oi in range(NOWN):
                        pp = psum.tile([P, 512], FP32, tag="mm", bufs=2)
                        for kk in range(FFT):
                            nc.tensor.matmul(pp, w2_blk[:, kk, :],
                                             m_sb[:, oi, kk, :],
                                             start=(kk == 0), stop=(kk == FFT - 1))
                        ost = mlp.tile([P, 512], FP32, tag="ost", bufs=2)
                        nc.vector.scalar_tensor_tensor(
                            ost, pp, b2_sb[:, ot:ot + 1], x2_sb[:, oi, ot, :],
                            ALU.add, ALU.add)
                        nc.sync.dma_start(out[:, oi, ot, :], ost)

    nc.compile()
    return nc


_NC_CACHE = None


def _get_nc():
    global _NC_CACHE
    if _NC_CACHE is None:
        _NC_CACHE = build_program(debug=bool(int(os.environ.get("KERNEL_DEBUG", "0"))))
    return _NC_CACHE


def prep_in_maps(x, ln1_g, ln1_b, ln2_g, ln2_b, Wq, bq, Wk, bk, Wv, bv,
                 Wo, bo, W1, b1, W2, b2):
    f32 = np.float32
    x = np.asarray(x, f32)
    ln1_g, ln1_b = np.asarray(ln1_g, f32), np.asarray(ln1_b, f32)
    ln2_g, ln2_b = np.asarray(ln2_g, f32), np.asarray(ln2_b, f32)
    Wq, Wk, Wv, Wo_ = (np.asarray(a, f32) for a in (Wq, Wk, Wv, Wo))
    W1, W2 = np.asarray(W1, f32), np.asarray(W2, f32)
    bq, bk, bv, bo_, b1, b2_ = (np.asarray(a, f32) for a in (bq, bk, bv, bo, b1, b2))

    # fold LN gain + mean subtraction (rank-1) into W; rstd applied to input
    def fold(g, W):
        Wg = g[:, None] * W
        return Wg - Wg.sum(0, keepdims=True) / D

    Wqg, Wkg, Wvg = fold(ln1_g, Wq), fold(ln1_g, Wk), fold(ln1_g, Wv)
    W1g = fold(ln2_g, W1)
    cq_full = ln1_b @ Wq + bq
    ck_full = ln1_b @ Wk + bk
    cv_full = ln1_b @ Wv + bv
    c1_full = ln2_b @ W1 + b1

    def wtile(W, kdim, odim):
        # [K, O] -> [O//P, P(row-within-ktile), K//P, P] block layout
        return np.ascontiguousarray(
            W.reshape(kdim // P, P, odim // P, P).transpose(2, 1, 0, 3)
        ).astype(NPBF)

    wk_t = wtile(Wkg, D, D)             # [DK, P, DK, P]
    wq_t = wtile(Wqg, D, D)             # [NP, P, DK, P] (NP == DK)
    wo_t = wtile(Wo_, D, D)
    w1_t = wtile(W1g, D, DFF)           # [FFT, P, DK, P]
    w2_t = wtile(W2, DFF, D)            # [DK, P, FFT, P]
    wv_t = np.ascontiguousarray(
        Wvg.reshape(DK, P, D).transpose(1, 0, 2)).astype(NPBF)  # [P, DK, D]

    ck_t = np.ascontiguousarray(ck_full.reshape(DK, P).T)
    cq_t = np.ascontiguousarray(cq_full.reshape(DK, P).T)
    ckq_t = np.concatenate([ck_t, cq_t], axis=1)                 # [P, 2DK]
    cvb_t = np.broadcast_to(cv_full[None, :], (P, D)).copy()
    bo_t = np.ascontiguousarray(bo_.reshape(DK, P).T)
    c1_t = np.ascontiguousarray(c1_full.reshape(FFT, P).T)
    b2_t = np.ascontiguousarray(b2_.reshape(DK, P).T)

    pp_ = np.arange(P)[:, None]
    cc_ = np.arange(512)[None, :]

    in_maps = []
    for c in range(8):
        b_idx, par = c // 2, c % 2
        slots = SLOT_TILES[par]
        arr4 = np.ascontiguousarray(x[b_idx].T).reshape(DK, P, NT, 512)
        xbf_c = np.ascontiguousarray(
            arr4[:, :, slots, :].transpose(1, 2, 0, 3)).astype(NPBF)
        xf32_c = np.ascontiguousarray(
            arr4[:, :, [slots[1], slots[3]], :].transpose(1, 2, 0, 3))
        mk = np.zeros((2, P, 8, 512), np.float32)
        for pi in range(2):
            q_tile = slots[Q_SLOTS[pi]]
            for j in range(8):
                blk = j if pi == 0 else 8 + j
                k_tile = slots[blk // 4]
                k_tok = 512 * k_tile + 128 * (blk % 4) + pp_
                q_tok = 512 * q_tile + cc_
                mk[pi, :, j, :] = (k_tok <= q_tok).astype(np.float32)
        in_maps.append({
            "xbf": xbf_c,
            "xf32": xf32_c,
            "wk": wk_t, "wv": wv_t, "wq": wq_t, "wo": wo_t,
            "w1": w1_t, "w2": w2_t,
            "masks": mk.astype(NPBF),
            "ckq": ckq_t, "cvb": cvb_t, "bo": bo_t, "c1": c1_t, "b2": b2_t,
        })
    return in_maps


def assemble_output(results):
    out = np.empty((B, T, D), np.float32)
    for c in range(8):
        b_idx, par = c // 2, c % 2
        slots = SLOT_TILES[par]
        o = results[c]["out"]  # [P, NOWN, DK, 512]
        for oi in range(NOWN):
            t0 = 512 * slots[Q_SLOTS[oi]]
            # out[b, t0+cc, 128*ot+p] = o[p, oi, ot, cc]
            out[b_idx, t0:t0 + 512, :] = (
                o[:, oi, :, :].transpose(2, 1, 0).reshape(512, D))
    return out


def kernel(**inputs):
    nc = _get_nc()
    in_maps = prep_in_maps(**inputs)
    res = run_bass_kernel_spmd(nc, in_maps, list(range(8)))
    return assemble_output(res.results)


# revision 6
# speedup vs baseline: 1.5697x; 1.2786x over previous
"""Trainium2 Bass kernel for a dense transformer block (B=4, T=2048, D=1024, H=16).

Sharding v2: 8 cores = 4 batches x 2 token-parities; no collectives.
Core (b, par) owns query tiles {0,3} (par=0) or {1,2} (par=1) of 512 tokens
(both = 5 causal units, balanced).  Every core computes LN1 + k/v projections
for all 4 key tiles of its batch (redundant within a pair), q for its own
tiles, full causal attention + complete Wo for its own tokens, then LN2 + MLP
locally.  The compiled program is identical on all cores (SPMD); per-core
differences live purely in DRAM data.

Uniformity trick: the host stages the four 512-token x tiles in per-core SLOT
order (par=0: tiles [1,0,2,3]; par=1: [0,1,3,2]) so that the two own q tiles
always sit at slots 1 and 3.  k/v are computed per slot; attention pass0
(q = slot1) runs over key blocks 0..7 (slots 0-1) and pass1 (q = slot3) over
blocks 0..15, with per-core causal masks (DRAM data) zeroing whatever a core
must not see.  Diagonal blocks (pass0: 4..7, pass1: 12..15 — the diag slot on
both cores) process only columns >= 128*j (exact shrink).

All matmul operands are bf16 (fp32 PSUM).  LayerNorm gain + mean subtraction
(rank-1) are folded into weights host-side; rstd is applied to the layer
input (xs = x * rstd) so projection epilogues are one bias-add+convert pass.
"""

import os
import sys

for _p in ("/opt/trn_rl_repo", "/root/.axon_site/_ro/trn_rl_repo"):
    if os.path.isdir(_p) and _p not in sys.path:
        sys.path.append(_p)

import ml_dtypes
import numpy as np

import concourse.bass as bass
import concourse.tile as tile
from concourse import bacc, mybir
from concourse.bass_utils import run_bass_kernel_spmd

AF = mybir.ActivationFunctionType
ALU = mybir.AluOpType
FP32 = mybir.dt.float32
BF16 = mybir.dt.bfloat16
NPBF = ml_dtypes.bfloat16

B, T, D, H = 4, 2048, 1024, 16
HD = D // H          # 64
DFF = 4 * D          # 4096
P = 128
DK = D // P          # 8   D k-tiles
NT = T // 512        # 4   512-token tiles
NP = H // 2          # 8   head pairs
FFT = DFF // P       # 32  DFF tiles
TC = T // P          # 16  128-token chunks
EPS = 1e-5
SCALE = 1.0 / 8.0    # 1/sqrt(HD)
NOWN = 2             # own 512-token tiles per core
NBLK = (8, 16)       # key blocks per attention pass
Q_SLOTS = (1, 3)     # slots holding the own q tiles

# slot -> tile permutations per parity
SLOT_TILES = ([1, 0, 2, 3], [0, 1, 3, 2])


def _shrink(pi, kt):
    if pi == 0 and kt >= 4:
        return 128 * (kt - 4)
    if pi == 1 and kt >= 12:
        return 128 * (kt - 12)
    return 0


def _masked(pi, kt):
    return kt >= 8 if pi == 1 else True


def build_program(debug=False, phases=("att", "mlp")):
    nc = bacc.Bacc("TRN2", target_bir_lowering=False, debug=False)

    # ---- DRAM I/O ----
    xbf = nc.dram_tensor("xbf", [P, NT, DK, 512], BF16, kind="ExternalInput")
    xf32 = nc.dram_tensor("xf32", [P, NOWN, DK, 512], FP32, kind="ExternalInput")
    wk = nc.dram_tensor("wk", [DK, P, DK, P], BF16, kind="ExternalInput")
    wv = nc.dram_tensor("wv", [P, DK, D], BF16, kind="ExternalInput")
    wq = nc.dram_tensor("wq", [NP, P, DK, P], BF16, kind="ExternalInput")
    wo = nc.dram_tensor("wo", [DK, P, NP, P], BF16, kind="ExternalInput")
    w1 = nc.dram_tensor("w1", [FFT, P, DK, P], BF16, kind="ExternalInput")
    w2 = nc.dram_tensor("w2", [DK, P, FFT, P], BF16, kind="ExternalInput")
    masks = nc.dram_tensor("masks", [2, P, 8, 512], BF16, kind="ExternalInput")
    ckq = nc.dram_tensor("ckq", [P, 2 * DK], FP32, kind="ExternalInput")  # [ck|cq]
    cvb = nc.dram_tensor("cvb", [P, D], FP32, kind="ExternalInput")
    bo = nc.dram_tensor("bo", [P, DK], FP32, kind="ExternalInput")
    c1 = nc.dram_tensor("c1", [P, FFT], FP32, kind="ExternalInput")
    b2 = nc.dram_tensor("b2", [P, DK], FP32, kind="ExternalInput")
    out = nc.dram_tensor("out", [P, NOWN, DK, 512], FP32, kind="ExternalOutput")
    if debug:
        dbg_kT = nc.dram_tensor("dbg_kT", [P, NP, T], BF16, kind="ExternalOutput")
        dbg_v = nc.dram_tensor("dbg_v", [P, H, TC, HD + 1], BF16,
                               kind="ExternalOutput")
        dbg_q = nc.dram_tensor("dbg_q", [P, NOWN, NP, 512], BF16,
                               kind="ExternalOutput")
        dbg_x2 = nc.dram_tensor("dbg_x2", [P, NOWN, DK, 512], BF16,
                                kind="ExternalOutput")
        dbg_y = nc.dram_tensor("dbg_y", [P, NOWN, NP, 512], BF16,
                               kind="ExternalOutput")

    with tile.TileContext(nc) as tc:
        with (
            tc.tile_pool(name="persist", bufs=1) as persist,
            tc.tile_pool(name="psum", bufs=1, space="PSUM") as psum,
        ):
            # ---- persistent small tiles ----
            ones_bf = persist.tile([P, 1], BF16)
            nc.vector.memset(ones_bf, 1.0)
            ckq_sb = persist.tile([P, 2 * DK], FP32)
            nc.sync.dma_start(ckq_sb, ckq[:, :])
            cvb_sb = persist.tile([P, D], FP32)
            nc.sync.dma_start(cvb_sb, cvb[:, :])
            bo_sb = persist.tile([P, DK], FP32)
            nc.sync.dma_start(bo_sb, bo[:, :])
            c1_sb = persist.tile([P, FFT], FP32)
            nc.sync.dma_start(c1_sb, c1[:, :])
            b2_sb = persist.tile([P, DK], FP32)
            nc.sync.dma_start(b2_sb, b2[:, :])
            eps_sb = persist.tile([1, 1], FP32)
            nc.vector.memset(eps_sb, EPS)
            # preload ACT function tables so no LoadActFuncSet lands mid-stream
            warm = persist.tile([1, 1], FP32)
            nc.scalar.activation(warm, eps_sb, AF.Sqrt)
            nc.scalar.activation(warm, eps_sb, AF.Exp)
            nc.scalar.activation(warm, eps_sb, AF.Gelu)
            row_a = persist.tile([1, 512], FP32)
            row_b = persist.tile([1, 512], FP32)
            row_mu = persist.tile([1, 512], FP32)
            x2_sb = persist.tile([P, NOWN, DK, 512], BF16)
            x2s_sb = persist.tile([P, NOWN, DK, 512], BF16)

            def ln_rows(s_ps, q_ps, rs_bf):
                nc.vector.tensor_scalar(row_mu, s_ps, 1.0 / D, None, ALU.mult)
                nc.vector.tensor_scalar(row_a, q_ps, 1.0 / D, None, ALU.mult)
                nc.vector.tensor_mul(row_b, row_mu, row_mu)
                nc.vector.tensor_sub(row_a, row_a, row_b)
                nc.scalar.activation(row_b, row_a, AF.Sqrt, bias=eps_sb)
                with nc.allow_low_precision(reason="bf16 rstd by design"):
                    nc.vector.reciprocal(rs_bf, row_b)

            with tc.tile_pool(name="work", bufs=1) as work:
              with tc.tile_pool(name="attn", bufs=1) as attn:
                wv_sb = attn.tile([P, DK, D], BF16)
                kT_sb = attn.tile([P, NP, T], BF16)        # [64*hb+e, pair, ktok]
                v_sb = attn.tile([P, H, TC, HD + 1], BF16)  # [ktok%128, h, kchunk, e|1]
                nc.vector.memset(v_sb[:, :, :, HD:HD + 1], 1.0)
                q_sb = attn.tile([P, NOWN, NP, 512], BF16)
                ysb = work.tile([P, NP, 512], BF16, tag="ysbt", bufs=1)

                def emit_den(y_ps, pt):
                    """Normalize y_ps into ysb[:, pt, :] (reciprocal of the
                    ones-column denominator, broadcast, multiply)."""
                    for hb in range(2):
                        den = work.tile([HD + 1, 512], FP32, tag="den", bufs=2)
                        nc.vector.reciprocal(den[HD:HD + 1, :],
                                             y_ps[hb][HD:HD + 1, :])
                        rec = work.tile([1, 512], FP32, tag="rec", bufs=2)
                        nc.sync.dma_start(rec, den[HD:HD + 1, :])
                        rb = work.tile([HD, 512], FP32, tag="rb", bufs=2)
                        nc.gpsimd.partition_broadcast(rb, rec)
                        if hb == 0:
                            nc.vector.tensor_mul(ysb[0:HD, pt, :],
                                                 y_ps[hb][0:HD, :], rb)
                        else:
                            yst = work.tile([HD, 512], BF16, tag="yst", bufs=2)
                            nc.vector.tensor_mul(yst, y_ps[hb][0:HD, :], rb)
                            nc.sync.dma_start(ysb[HD:2 * HD, pt, :], yst)

                def attention(pi, fillers=(), pops=2):
                    fillers = list(fillers)
                    nblk = NBLK[pi]
                    mask_sb = work.tile([P, 8, 512], BF16, tag="mask", bufs=1)
                    nc.sync.dma_start(mask_sb, masks[pi])
                    prev_y = None
                    for pt in range(NP):
                        for _ in range(pops):
                            if fillers:
                                fillers.pop(0)()
                        y_ps = [psum.tile([HD + 1, 512], FP32, name=f"yps{hb}",
                                          tag="y", bufs=2) for hb in range(2)]
                        for kt in range(nblk):
                            off = _shrink(pi, kt)
                            pexp = []
                            for hb in range(2):
                                hsl = slice(hb * HD, (hb + 1) * HD)
                                s_ps = psum.tile([P, 512], FP32, tag="mm", bufs=2)
                                nc.tensor.matmul(
                                    s_ps[:, off:],
                                    kT_sb[hsl, pt, kt * P:(kt + 1) * P],
                                    q_sb[hsl, pi, pt, off:], start=True, stop=True)
                                pe = work.tile([P, 512], BF16, tag="pexp", bufs=4)
                                nc.scalar.activation(pe[:, off:], s_ps[:, off:],
                                                     AF.Exp, scale=SCALE)
                                if _masked(pi, kt):
                                    mi = kt if pi == 0 else kt - 8
                                    nc.vector.tensor_mul(pe[:, off:], pe[:, off:],
                                                         mask_sb[:, mi, off:])
                                pexp.append(pe)
                            # normalize the previous pair's y while this one's
                            # exp/AV chain runs (keeps the DVE den chain out of
                            # the critical path)
                            if kt == 1 and prev_y is not None:
                                emit_den(prev_y, pt - 1)
                            for hb in range(2):
                                nc.tensor.matmul(
                                    y_ps[hb][:, off:], v_sb[:, 2 * pt + hb, kt, :],
                                    pexp[hb][:, off:],
                                    start=(kt == 0), stop=(kt == nblk - 1))
                        prev_y = y_ps
                    emit_den(prev_y, NP - 1)
                    if debug:
                        nc.sync.dma_start(dbg_y[:, pi], ysb)
                    # ---- Wo + x2 + LN2 stats ----
                    s2_ps = psum.tile([1, 512], FP32, tag="st", bufs=2)
                    q2_ps = psum.tile([1, 512], FP32, tag="st", bufs=2)
                    for ot in range(DK):
                        wo_blk = work.tile([P, NP, P], BF16, tag="wob", bufs=2)
                        nc.sync.dma_start(wo_blk, wo[ot])
                        xo_blk = work.tile([P, 512], FP32, tag="xob", bufs=1)
                        nc.sync.dma_start(xo_blk, xf32[:, pi, ot, :])
                        pp = psum.tile([P, 512], FP32, tag="mm", bufs=2)
                        for pt in range(NP):
                            nc.tensor.matmul(pp, wo_blk[:, pt, :], ysb[:, pt, :],
                                             start=(pt == 0), stop=(pt == NP - 1))
                        nc.vector.scalar_tensor_tensor(
                            x2_sb[:, pi, ot, :], pp, bo_sb[:, ot:ot + 1], xo_blk,
                            ALU.add, ALU.add)
                        nc.tensor.matmul(s2_ps, ones_bf, x2_sb[:, pi, ot, :],
                                         start=(ot == 0), stop=(ot == DK - 1))
                        xsq = work.tile([P, 512], BF16, tag="xsq", bufs=1)
                        nc.vector.tensor_mul(xsq, x2_sb[:, pi, ot, :],
                                             x2_sb[:, pi, ot, :])
                        nc.tensor.matmul(q2_ps, ones_bf, xsq,
                                         start=(ot == 0), stop=(ot == DK - 1))
                    rs2_bf = work.tile([1, 512], BF16, tag="rsrow", bufs=2)
                    ln_rows(s2_ps, q2_ps, rs2_bf)
                    rsb2 = work.tile([P, 512], BF16, tag="rsb", bufs=2)
                    nc.gpsimd.partition_broadcast(rsb2, rs2_bf)
                    for kt in range(DK):
                        nc.vector.tensor_mul(x2s_sb[:, pi, kt, :],
                                             x2_sb[:, pi, kt, :], rsb2)

                # ==== per 512-token slot: LN1 stats, k/v (+q); attention ====
                def load_and_stats(tt):
                    """DMA x tile, LN1 stats, then scale it in place by rstd."""
                    xt = work.tile([P, DK, 512], BF16, tag="xt", bufs=2)
                    nc.sync.dma_start(xt, xbf[:, tt])
                    s_ps = psum.tile([1, 512], FP32, tag="st", bufs=2)
                    q_ps = psum.tile([1, 512], FP32, tag="st", bufs=2)
                    for kt in range(DK):
                        nc.tensor.matmul(s_ps, ones_bf, xt[:, kt, :],
                                         start=(kt == 0), stop=(kt == DK - 1))
                    for kt in range(DK):
                        xsq = work.tile([P, 512], BF16, tag="xsq", bufs=1)
                        nc.vector.tensor_mul(xsq, xt[:, kt, :], xt[:, kt, :])
                        nc.tensor.matmul(q_ps, ones_bf, xsq,
                                         start=(kt == 0), stop=(kt == DK - 1))
                    rs_bf = work.tile([1, 512], BF16, tag="rsrow", bufs=2)
                    ln_rows(s_ps, q_ps, rs_bf)
                    rsb = work.tile([P, 512], BF16, tag="rsb", bufs=2)
                    nc.gpsimd.partition_broadcast(rsb, rs_bf)
                    for kt in range(DK):
                        nc.vector.tensor_mul(xt[:, kt, :], xt[:, kt, :], rsb)
                    return xt

                def kproj_unit(xs, tt, ot):
                    wk_blk = work.tile([P, DK, P], BF16, tag="wkb", bufs=2)
                    nc.sync.dma_start(wk_blk, wk[ot])
                    pp = psum.tile([P, 512], FP32, tag="mm", bufs=2)
                    for kt in range(DK):
                        nc.tensor.matmul(pp, wk_blk[:, kt, :], xs[:, kt, :],
                                         start=(kt == 0), stop=(kt == DK - 1))
                    nc.vector.tensor_scalar(
                        kT_sb[:, ot, tt * 512:(tt + 1) * 512], pp,
                        ckq_sb[:, ot:ot + 1], None, ALU.add)

                def vproj_unit(xs, tt, st):
                    # halves sequential: holds only one psum slot at a time so
                    # interleaved attention scores keep their ping-pong buffer
                    for half in range(2):
                        pv = psum.tile([P, 512], FP32, tag="mm", bufs=2)
                        for kt in range(DK):
                            nc.tensor.matmul(
                                pv, xs[:, kt, st * P:(st + 1) * P],
                                wv_sb[:, kt, half * 512:(half + 1) * 512],
                                start=(kt == 0), stop=(kt == DK - 1))
                        dest = v_sb[:, half * NP:(half + 1) * NP, tt * 4 + st, 0:HD]
                        nc.vector.tensor_tensor(
                            dest,
                            pv.rearrange("p (h e) -> p h e", h=NP),
                            cvb_sb[:, half * 512:(half + 1) * 512]
                            .rearrange("p (h e) -> p h e", h=NP), ALU.add)

                def qproj_unit(xs, oi, ot):
                    wq_blk = work.tile([P, DK, P], BF16, tag="wqb", bufs=2)
                    nc.sync.dma_start(wq_blk, wq[ot])
                    pp = psum.tile([P, 512], FP32, tag="mm", bufs=2)
                    for kt in range(DK):
                        nc.tensor.matmul(pp, wq_blk[:, kt, :], xs[:, kt, :],
                                         start=(kt == 0), stop=(kt == DK - 1))
                    nc.vector.tensor_scalar(
                        q_sb[:, oi, ot, :], pp,
                        ckq_sb[:, DK + ot:DK + ot + 1], None, ALU.add)

                xs0 = load_and_stats(0)
                nc.sync.dma_start(wv_sb, wv[:, :, :])
                for ot in range(DK):
                    kproj_unit(xs0, 0, ot)
                xs1 = load_and_stats(1)
                for st in range(4):
                    vproj_unit(xs0, 0, st)
                for ot in range(DK):
                    kproj_unit(xs1, 1, ot)
                xs2 = load_and_stats(2)
                for st in range(4):
                    vproj_unit(xs1, 1, st)
                for ot in range(NP):
                    qproj_unit(xs1, 0, ot)
                # attention pass0 interleaved with slot2 projections + slot3
                # stats (fills the PE during the ACT-bound exp stretches)
                holder = {}
                fillers = (
                    [lambda ot=ot: kproj_unit(xs2, 2, ot) for ot in range(DK)]
                    + [lambda: holder.__setitem__("xs3", load_and_stats(3))]
                    + [lambda st=st: vproj_unit(xs2, 2, st) for st in range(4)]
                )
                if "att" in phases:
                    attention(0, fillers)
                else:
                    for f in fillers:
                        f()
                xs3 = holder["xs3"]
                for st in range(4):
                    vproj_unit(xs3, 3, st)
                for ot in range(NP):
                    qproj_unit(xs3, 1, ot)
                fillers1 = [lambda ot=ot: kproj_unit(xs3, 3, ot)
                            for ot in range(DK)]
                if "att" in phases:
                    attention(1, fillers1, pops=1)
                else:
                    for f in fillers1:
                        f()

                if debug:
                    nc.sync.dma_start(dbg_kT[:, :, :], kT_sb)
                    nc.sync.dma_start(dbg_v[:, :, :, :], v_sb)
                    nc.sync.dma_start(dbg_q[:, :, :, :], q_sb)
                    nc.sync.dma_start(dbg_x2[:, :, :, :], x2_sb)

              # ==== MLP over both own tiles (reuses the attn pool space;
              # fc1 for own-tile 0 overlaps the Wo/LN2 tail of pass 1) ====
              with tc.tile_pool(name="mlp", bufs=1) as mlp:
                if "mlp" in phases:
                  m_sb = mlp.tile([P, NOWN, FFT, 512], BF16)
                  for oi in range(NOWN):
                    for fft in range(FFT):
                        w1_blk = mlp.tile([P, DK, P], BF16, tag="w1b", bufs=3)
                        nc.sync.dma_start(w1_blk, w1[fft])
                        pp = psum.tile([P, 512], FP32, tag="mm", bufs=2)
                        for kt in range(DK):
                            nc.tensor.matmul(pp, w1_blk[:, kt, :],
                                             x2s_sb[:, oi, kt, :],
                                             start=(kt == 0), stop=(kt == DK - 1))
                        nc.scalar.activation(m_sb[:, oi, fft, :], pp, AF.Gelu,
                                             bias=c1_sb[:, fft:fft + 1])
                  for ot in range(DK):
                    w2_blk = mlp.tile([P, FFT, P], BF16, tag="w2b", bufs=2)
                    nc.sync.dma_start(w2_blk, w2[ot])
                    for oi in range(NOWN):
                        pp = psum.tile([P, 512], FP32, tag="mm", bufs=2)
                        for kk in range(FFT):
                            nc.tensor.matmul(pp, w2_blk[:, kk, :],
                                             m_sb[:, oi, kk, :],
                                             start=(kk == 0), stop=(kk == FFT - 1))
                        ost = mlp.tile([P, 512], FP32, tag="ost", bufs=2)
                        nc.vector.scalar_tensor_tensor(
                            ost, pp, b2_sb[:, ot:ot + 1], x2_sb[:, oi, ot, :],
                            ALU.add, ALU.add)
                        nc.sync.dma_start(out[:, oi, ot, :], ost)

    nc.compile()
    return nc


_NC_CACHE = None


def _get_nc():
    global _NC_CACHE
    if _NC_CACHE is None:
        _NC_CACHE = build_program(debug=bool(int(os.environ.get("KERNEL_DEBUG", "0"))))
    return _NC_CACHE


def prep_in_maps(x, ln1_g, ln1_b, ln2_g, ln2_b, Wq, bq, Wk, bk, Wv, bv,
                 Wo, bo, W1, b1, W2, b2):
    f32 = np.float32
    x = np.asarray(x, f32)
    ln1_g, ln1_b = np.asarray(ln1_g, f32), np.asarray(ln1_b, f32)
    ln2_g, ln2_b = np.asarray(ln2_g, f32), np.asarray(ln2_b, f32)
    Wq, Wk, Wv, Wo_ = (np.asarray(a, f32) for a in (Wq, Wk, Wv, Wo))
    W1, W2 = np.asarray(W1, f32), np.asarray(W2, f32)
    bq, bk, bv, bo_, b1, b2_ = (np.asarray(a, f32) for a in (bq, bk, bv, bo, b1, b2))

    # fold LN gain + mean subtraction (rank-1) into W; rstd applied to input
    def fold(g, W):
        Wg = g[:, None] * W
        return Wg - Wg.sum(0, keepdims=True) / D

    Wqg, Wkg, Wvg = fold(ln1_g, Wq), fold(ln1_g, Wk), fold(ln1_g, Wv)
    W1g = fold(ln2_g, W1)
    cq_full = ln1_b @ Wq + bq
    ck_full = ln1_b @ Wk + bk
    cv_full = ln1_b @ Wv + bv
    c1_full = ln2_b @ W1 + b1

    def wtile(W, kdim, odim):
        # [K, O] -> [O//P, P(row-within-ktile), K//P, P] block layout
        return np.ascontiguousarray(
            W.reshape(kdim // P, P, odim // P, P).transpose(2, 1, 0, 3)
        ).astype(NPBF)

    wk_t = wtile(Wkg, D, D)             # [DK, P, DK, P]
    wq_t = wtile(Wqg, D, D)             # [NP, P, DK, P] (NP == DK)
    wo_t = wtile(Wo_, D, D)
    w1_t = wtile(W1g, D, DFF)           # [FFT, P, DK, P]
    w2_t = wtile(W2, DFF, D)            # [DK, P, FFT, P]
    wv_t = np.ascontiguousarray(
        Wvg.reshape(DK, P, D).transpose(1, 0, 2)).astype(NPBF)  # [P, DK, D]

    ck_t = np.ascontiguousarray(ck_full.reshape(DK, P).T)
    cq_t = np.ascontiguousarray(cq_full.reshape(DK, P).T)
    ckq_t = np.concatenate([ck_t, cq_t], axis=1)                 # [P, 2DK]
    cvb_t = np.broadcast_to(cv_full[None, :], (P, D)).copy()
    bo_t = np.ascontiguousarray(bo_.reshape(DK, P).T)
    c1_t = np.ascontiguousarray(c1_full.reshape(FFT, P).T)
    b2_t = np.ascontiguousarray(b2_.reshape(DK, P).T)

    pp_ = np.arange(P)[:, None]
    cc_ = np.arange(512)[None, :]

    in_maps = []
    for c in range(8):
        b_idx, par = c // 2, c % 2
        slots = SLOT_TILES[par]
        arr4 = np.ascontiguousarray(x[b_idx].T).reshape(DK, P, NT, 512)
        xbf_c = np.ascontiguousarray(
            arr4[:, :, slots, :].transpose(1, 2, 0, 3)).astype(NPBF)
        xf32_c = np.ascontiguousarray(
            arr4[:, :, [slots[1], slots[3]], :].transpose(1, 2, 0, 3))
        mk = np.zeros((2, P, 8, 512), np.float32)
        for pi in range(2):
            q_tile = slots[Q_SLOTS[pi]]
            for j in range(8):
                blk = j if pi == 0 else 8 + j
                k_tile = slots[blk // 4]
                k_tok = 512 * k_tile + 128 * (blk % 4) + pp_
                q_tok = 512 * q_tile + cc_
                mk[pi, :, j, :] = (k_tok <= q_tok).astype(np.float32)
        in_maps.append({
            "xbf": xbf_c,
            "xf32": xf32_c,
            "wk": wk_t, "wv": wv_t, "wq": wq_t, "wo": wo_t,
            "w1": w1_t, "w2": w2_t,
            "masks": mk.astype(NPBF),
            "ckq": ckq_t, "cvb": cvb_t, "bo": bo_t, "c1": c1_t, "b2": b2_t,
        })
    return in_maps


def assemble_output(results):
    out = np.empty((B, T, D), np.float32)
    for c in range(8):
        b_idx, par = c // 2, c % 2
        slots = SLOT_TILES[par]
        o = results[c]["out"]  # [P, NOWN, DK, 512]
        for oi in range(NOWN):
            t0 = 512 * slots[Q_SLOTS[oi]]
            # out[b, t0+cc, 128*ot+p] = o[p, oi, ot, cc]
            out[b_idx, t0:t0 + 512, :] = (
                o[:, oi, :, :].transpose(2, 1, 0).reshape(512, D))
    return out


def kernel(**inputs):
    nc = _get_nc()
    in_maps = prep_in_maps(**inputs)
    res = run_bass_kernel_spmd(nc, in_maps, list(range(8)))
    return assemble_output(res.results)
